# revision 1
# baseline (speedup 1.0000x reference)
"""Trainium2 Bass kernel for a GAT block (GATConv + LN + FFN + LN).

Self-contained: builds per-core shards on the host, compiles one SPMD Bass
program, runs it on 8 NeuronCores via run_bass_kernel_spmd, and reassembles
the full [50000, 128] output.

Per-core scheme (core c of 8, nodes permuted own-first per core):
  Phase A: h' = x @ [W | W@Adst] for all 50176 (padded) nodes; h rows (bf16)
           stored to core-local DRAM for gathering; a_dst kept on-chip,
           replicated across partitions for ap_gather.
  Phase B: edges with dst owned by the core (incl self-loops), grouped by
           128-node block, split lo/hi on the 32K int16 gather-index limit,
           padded per (block, stream) to 128-edge granules with a shared
           max-over-cores profile so all cores run one program.
           Per 4096-edge chunk: dma_gather h rows; a_src = seg16-reduce of
           h*att_src; a_dst via ap_gather + per-granule PE transposes;
           p = exp(leaky_relu(a_src + a_dst)); msg = h * expand16(p);
           S = (dst_in_block == iota); per granule: psum[block] +=
           S^T @ [msg | p] (one matmul: lhsT=S, rhs=[msg|p]).
  Phase C: g = agg/denom; u = LN(x + g); ff = relu(u@W1 + b1)@W2 + b2;
           z = LN(u + ff) -> output rows.
"""
import numpy as np
import ml_dtypes

N = 50000
NCORES = 8
OWN = 6272             # nodes per core (49 tiles of 128)
NP = OWN * NCORES      # padded node count
BLK = 128              # aggregation block == node tile
NBLK = OWN // BLK      # 49
GR = 128               # edges per granule
CHUNK = 4096           # edges per gather chunk (32 granules)
GPC = CHUNK // GR      # granules per chunk
LO_LIM = 1 << 15
H, F, D = 8, 16, 128
PAD_DL = 200.0         # sentinel dst_in_block for pad edges
LN_EPS = 1e-5

bf16 = ml_dtypes.bfloat16


def _wrap16(idx):
    L = idx.shape[0]
    w = idx.reshape(L // 16, 16).T.astype(np.int16)
    return np.tile(w, (8, 1))                      # [128, L/16]


def _bfr(x):
    return np.ascontiguousarray(x, dtype=np.float32).astype(bf16)


def _build_host_data(inputs):
    x = np.asarray(inputs["x"], np.float32)
    W = np.asarray(inputs["W_gat"], np.float32)
    att_src = np.asarray(inputs["att_src"], np.float32)
    att_dst = np.asarray(inputs["att_dst"], np.float32)
    ei = np.asarray(inputs["edge_index"])

    src = ei[0].astype(np.int64)
    dst = ei[1].astype(np.int64)
    loops = np.arange(N, dtype=np.int64)
    src = np.concatenate([src, loops])
    dst = np.concatenate([dst, loops])

    # per-core own-first permutation; row index of global node n on core c:
    #   own nodes -> [0, OWN); others keep relative order after them
    perms = []      # perms[c][row] = global node (pad rows -> n >= N map to 0-fill)
    invs = []       # invs[c][global padded node] = row
    allp = np.arange(NP, dtype=np.int64)
    for c in range(NCORES):
        own = allp[OWN * c: OWN * (c + 1)]
        rest = np.concatenate([allp[: OWN * c], allp[OWN * (c + 1):]])
        perm = np.concatenate([own, rest])
        inv = np.empty(NP, dtype=np.int64)
        inv[perm] = np.arange(NP)
        perms.append(perm)
        invs.append(inv)

    # per (core, block, stream) counts on permuted gather indices
    counts = np.zeros((NCORES, NBLK, 2), dtype=np.int64)
    core_edges = []
    for c in range(NCORES):
        m = (dst >= OWN * c) & (dst < min(OWN * (c + 1), N))
        s_g = invs[c][src[m]]                     # permuted gather row
        d_l = dst[m] - OWN * c                    # own-local dst == row (own-first)
        blk = d_l // BLK
        lo = s_g < LO_LIM
        core_edges.append((s_g, d_l, blk, lo))
        for b in range(NBLK):
            mb = blk == b
            counts[c, b, 0] = np.sum(mb & lo)
            counts[c, b, 1] = np.sum(mb & ~lo)

    g_prof = np.ceil(counts.max(axis=0) / GR).astype(np.int64)   # [NBLK, 2]
    L = [int(g_prof[:, s].sum()) * GR for s in range(2)]
    for s in range(2):
        pad = (-L[s]) % CHUNK
        g_prof[NBLK - 1, s] += pad // GR
        L[s] += pad
    L_LO, L_HI = L

    per_core = []
    for c in range(NCORES):
        s_g, d_l, blk, lo = core_edges[c]
        streams = []
        for sidx in range(2):
            mm = lo if sidx == 0 else ~lo
            Ls = L[sidx]
            gidx = np.zeros(Ls, dtype=np.int64)
            dl = np.full(Ls, PAD_DL, dtype=np.float32)
            al = np.zeros(Ls, dtype=np.int64)
            pos = 0
            for b in range(NBLK):
                mb = (blk == b) & mm
                k = int(np.sum(mb))
                cap = int(g_prof[b, sidx]) * GR
                gidx[pos:pos + k] = s_g[mb] - (0 if sidx == 0 else LO_LIM)
                dl[pos:pos + k] = (d_l[mb] % BLK).astype(np.float32)
                al[pos:pos + k] = d_l[mb]
                pos += cap
            streams.append({
                "gidx16": _wrap16(gidx),
                "aidx16": _wrap16(al),
                "dl": np.ascontiguousarray(
                    dl.astype(bf16).reshape(-1, GR).T),     # [128, L/128]
            })
        per_core.append(streams)

    # weights
    Adst = np.zeros((D, H), np.float32)
    for h in range(H):
        Adst[h * F:(h + 1) * F, h] = att_dst[h]
    Wp = _bfr(np.concatenate([W, W @ Adst], axis=1))             # [128,136] bf16
    attS = _bfr(np.tile(att_src.reshape(1, -1), (128, 1)))       # [128,128] bf16
    iota = _bfr(np.tile(np.arange(BLK, dtype=np.float32), (128, 1)))
    I128 = _bfr(np.eye(128, dtype=np.float32))

    xp = np.zeros((NP, D), np.float32)
    xp[:N] = x
    xT_per_core = []
    x_own_per_core = []
    for c in range(NCORES):
        xTc = np.ascontiguousarray(xp[perms[c]].T.astype(bf16))  # [128, NP] bf16
        xT_per_core.append(xTc)
        x_own_per_core.append(np.ascontiguousarray(xp[OWN * c: OWN * (c + 1)]))

    host = {
        "g_prof": g_prof, "L_LO": L_LO, "L_HI": L_HI,
        "per_core": per_core, "xT": xT_per_core, "x_own": x_own_per_core,
        "Wp": Wp, "attS": attS, "iota": iota, "I128": I128,
        "W1": _bfr(np.asarray(inputs["w_ff1"], np.float32)),     # [128,256]
        "W2": _bfr(np.asarray(inputs["w_ff2"], np.float32)),     # [256,128]
        "b1col": np.ascontiguousarray(
            np.asarray(inputs["b_ff1"], np.float32).reshape(2, 128).T),  # [128,2]
    }
    # general-path extras (applied only when nontrivial)
    host["bias_gat"] = np.asarray(inputs["bias_gat"], np.float32)
    host["b_ff2"] = np.asarray(inputs["b_ff2"], np.float32)
    for nm in ("gamma1", "beta1", "gamma2", "beta2"):
        host[nm] = np.asarray(inputs[nm], np.float32)
    host["triv_gb1"] = bool(np.all(host["gamma1"] == 1) and np.all(host["beta1"] == 0))
    host["triv_gb2"] = bool(np.all(host["gamma2"] == 1) and np.all(host["beta2"] == 0))
    host["triv_bgat"] = bool(np.all(host["bias_gat"] == 0))
    host["triv_bff2"] = bool(np.all(host["b_ff2"] == 0))
    return host


def _build_program(host, phases="ABC", bstep=5):
    import concourse.bacc as bacc
    import concourse.mybir as mybir
    import concourse.tile as tile
    from concourse.bass import AP

    fp32 = mybir.dt.float32
    bft = mybir.dt.bfloat16
    i16 = mybir.dt.int16
    Alu = mybir.AluOpType
    Act = mybir.ActivationFunctionType
    Axis = mybir.AxisListType

    g_prof = host["g_prof"]
    L_LO, L_HI = host["L_LO"], host["L_HI"]

    nc = bacc.Bacc("TRN2")

    # ---- DRAM tensors ----
    xT_d = nc.dram_tensor("xT", [128, NP], bft, kind="ExternalInput")
    xown_d = nc.dram_tensor("x_own", [OWN, D], fp32, kind="ExternalInput")
    Wp_d = nc.dram_tensor("Wp", [128, 136], bft, kind="ExternalInput")
    attS_d = nc.dram_tensor("attS", [128, 128], bft, kind="ExternalInput")
    iota_d = nc.dram_tensor("iota", [128, BLK], bft, kind="ExternalInput")
    I128_d = nc.dram_tensor("I128", [128, 128], bft, kind="ExternalInput")
    W1_d = nc.dram_tensor("W1", [128, 256], bft, kind="ExternalInput")
    W2_d = nc.dram_tensor("W2", [256, 128], bft, kind="ExternalInput")
    b1c_d = nc.dram_tensor("b1col", [128, 2], fp32, kind="ExternalInput")
    gl_d = {}
    if not host["triv_bgat"]:
        gl_d["bgat"] = nc.dram_tensor("bgat_r", [128, 128], fp32, kind="ExternalInput")
    if not host["triv_bff2"]:
        gl_d["bff2"] = nc.dram_tensor("bff2_r", [128, 128], fp32, kind="ExternalInput")
    if not host["triv_gb1"]:
        gl_d["g1"] = nc.dram_tensor("g1_r", [128, 128], fp32, kind="ExternalInput")
        gl_d["b1"] = nc.dram_tensor("b1_r", [128, 128], fp32, kind="ExternalInput")
    if not host["triv_gb2"]:
        gl_d["g2"] = nc.dram_tensor("g2_r", [128, 128], fp32, kind="ExternalInput")
        gl_d["b2"] = nc.dram_tensor("b2_r", [128, 128], fp32, kind="ExternalInput")

    st_d = []
    for sname, Ls in (("lo", L_LO), ("hi", L_HI)):
        st_d.append({
            "gidx": nc.dram_tensor(f"gidx_{sname}", [128, Ls // 16], i16,
                                   kind="ExternalInput"),
            "aidx": nc.dram_tensor(f"aidx_{sname}", [128, Ls // 16], i16,
                                   kind="ExternalInput"),
            "dl": nc.dram_tensor(f"dl_{sname}", [128, Ls // GR], bft,
                                 kind="ExternalInput"),
            "L": Ls,
        })

    h_d = nc.dram_tensor("h_scratch", [NP, D], bft, kind="Internal")
    ad_d = nc.dram_tensor("adst_scratch", [OWN, D], bft, kind="Internal")
    z_d = nc.dram_tensor("z", [OWN, D], fp32, kind="ExternalOutput")

    NT = NP // 128                    # 392 node tiles
    with tile.TileContext(nc) as tc:
        # ================= consts =================
        cpool = tc.alloc_tile_pool(name="consts", bufs=1)
        Wp_s = cpool.tile([128, 136], bft)
        nc.sync.dma_start(out=Wp_s[:], in_=Wp_d[:])
        attS_s = cpool.tile([128, 128], bft)
        nc.sync.dma_start(out=attS_s[:], in_=attS_d[:])
        iota_s = cpool.tile([128, BLK], bft)
        nc.sync.dma_start(out=iota_s[:], in_=iota_d[:])
        I128_s = cpool.tile([128, 128], bft)
        nc.sync.dma_start(out=I128_s[:], in_=I128_d[:])
        W1_s = cpool.tile([128, 256], bft)
        nc.sync.dma_start(out=W1_s[:], in_=W1_d[:])
        W2_s = cpool.tile([256 // 2, 2, 128], bft)   # [128, 2, 128]: chunk k rows
        nc.sync.dma_start(out=W2_s[:],
                          in_=W2_d[:].rearrange("(k h) f -> h k f", k=2))
        b1c_s = cpool.tile([128, 2], fp32)
        nc.sync.dma_start(out=b1c_s[:], in_=b1c_d[:])
        gl_s = {}
        for k, dref in gl_d.items():
            gl_s[k] = cpool.tile([128, 128], fp32, tag=f"gl_{k}")
            nc.sync.dma_start(out=gl_s[k][:], in_=dref[:])
        eps_s = cpool.tile([128, 1], fp32)
        nc.vector.memset(eps_s[:], LN_EPS)

        # ================= phase A =================
        with tc.tile_pool(name="pA", bufs=4) as pA, \
             tc.tile_pool(name="psA", bufs=4, space="PSUM") as psA:
            GT = 3                                   # node tiles per psum bank
            XB = 12                                  # node tiles per x DMA
            xt = None
            for tg in range((NT + GT - 1) // GT):
                t0 = tg * GT
                ntl = min(GT, NT - t0)
                if t0 % XB == 0:
                    nxb = min(XB, NT - t0)
                    xt = pA.tile([128, XB * 128], bft, tag="xt")
                    nc.sync.dma_start(out=xt[:, :nxb * 128],
                                      in_=xT_d[:, t0 * 128:(t0 + nxb) * 128])
                ps = psA.tile([128, GT, 136], fp32, tag="psA")
                for j in range(ntl):
                    jo = (t0 % XB) + j
                    nc.tensor.matmul(ps[:, j, :],
                                     lhsT=xt[:, jo * 128:(jo + 1) * 128],
                                     rhs=Wp_s[:], start=True, stop=True)
                stage = pA.tile([128, GT, 136], bft, tag="stage")
                eng = nc.scalar if tg % 2 == 0 else nc.vector
                if eng is nc.scalar:
                    nc.scalar.activation(out=stage[:, :ntl, :], in_=ps[:, :ntl, :],
                                         func=Act.Copy)
                else:
                    nc.vector.tensor_copy(out=stage[:, :ntl, :], in_=ps[:, :ntl, :])
                nc.sync.dma_start(
                    out=h_d[t0 * 128:(t0 + ntl) * 128, :].rearrange(
                        "(j n) d -> n j d", j=ntl),
                    in_=stage[:, :ntl, :128])
                # a_dst rows (8 bf16 each) to DRAM for own tiles
                for j in range(ntl):
                    t = t0 + j
                    if t >= NBLK:
                        continue
                    nc.sync.dma_start(out=ad_d[t * 128:(t + 1) * 128, 0:8],
                                      in_=stage[:, j, 128:136])

        tc.strict_bb_all_engine_barrier()

        # ================= phases B + C =================
        run_B = "B" in phases
        run_C = "C" in phases
        if run_B and bstep == 0:
            with tc.tile_pool(name="dbg", bufs=2) as dp:
                for t in range(NBLK):
                    dt_ = dp.tile([128, 128], bft, tag="d")
                    nc.sync.dma_start(out=dt_[:], in_=h_d[t * 128:(t + 1) * 128, :])
                    nc.gpsimd.dma_start(out=z_d[t * 128:(t + 1) * 128, :], in_=dt_[:])
            run_B = False
        h_lo = h_d[0:LO_LIM, :]
        h_hi = h_d[LO_LIM:NP, :]
        starts = np.zeros((NBLK, 2), dtype=np.int64)   # granule start per block
        for s in range(2):
            starts[1:, s] = np.cumsum(g_prof[:-1, s])

        pB = tc.alloc_tile_pool(name="pB", bufs=2)
        pBs = tc.alloc_tile_pool(name="pBsmall", bufs=4)
        psB = tc.alloc_tile_pool(name="psB", bufs=3, space="PSUM")
        pC = tc.alloc_tile_pool(name="pC", bufs=2)
        psC = tc.alloc_tile_pool(name="psC", bufs=1, space="PSUM")

        chunk_tiles = [{}, {}]        # per stream: chunk idx -> tiles

        def emit_chunk(s, k):
            if k in chunk_tiles[s]:
                return chunk_tiles[s][k]
            sd = st_d[s]
            gix = pBs.tile([128, CHUNK // 16], i16, tag=f"gix{s}")
            nc.sync.dma_start(out=gix[:],
                              in_=sd["gidx"][:, k * (CHUNK // 16):(k + 1) * (CHUNK // 16)])
            aix = pBs.tile([128, CHUNK // 16], i16, tag=f"aix{s}")
            nc.sync.dma_start(out=aix[:],
                              in_=sd["aidx"][:, k * (CHUNK // 16):(k + 1) * (CHUNK // 16)])
            dlt = pBs.tile([128, GPC], bft, tag=f"dl{s}")
            nc.sync.dma_start(out=dlt[:],
                              in_=sd["dl"][:, k * GPC:(k + 1) * GPC])
            h_ch = pB.tile([128, GPC, 128], bft, tag=f"h{s}")
            nc.gpsimd.dma_gather(h_ch[:], h_lo if s == 0 else h_hi, gix[:],
                                 CHUNK, CHUNK, 128, single_packet=False)
            msgp = pB.tile([128, GPC, 136], bft, tag=f"msgp{s}")
            if bstep < 2:
                if s == 0 and k == 0:
                    for g in range(GPC):
                        nc.gpsimd.dma_start(out=z_d[g * 128:(g + 1) * 128, :],
                                            in_=h_ch[:, g, :])
                res = {"S": None, "msgp": msgp, "h": h_ch}
                chunk_tiles[s][k] = res
                return res
            # a_src = seg16 reduce of h * attS (msgp body doubles as scratch)
            attS_b = AP(attS_s[:].tensor, attS_s[:].offset,
                        [attS_s[:].ap[0], [0, GPC], attS_s[:].ap[1]])
            nc.vector.tensor_tensor(out=msgp[:, :, 0:128], in0=h_ch[:], in1=attS_b,
                                    op=Alu.mult)
            asr = pBs.tile([128, GPC * 8], fp32, tag=f"asr{s}")
            nc.vector.tensor_reduce(
                out=asr[:],
                in_=msgp[:, :, 0:128].rearrange("p g (h f) -> p g h f", f=F),
                axis=Axis.X, op=Alu.add)
            # a_dst per edge: 256B-row gather from ad_d (edge-partition layout)
            adE = pB.tile([128, GPC, 128], bft, tag=f"adE{s}")
            nc.gpsimd.dma_gather(adE[:], ad_d[:], aix[:], CHUNK, CHUNK, 128,
                                 single_packet=False)
            if bstep < 3:
                if s == 0 and k == 0:
                    nc.sync.dma_start(out=z_d[0:128, :], in_=asr[:, 0:128])
                    nc.sync.dma_start(out=z_d[128:256, :], in_=asr[:, 128:256])
                    for g in range(8):
                        nc.gpsimd.dma_start(out=z_d[(4 + g) * 128:(5 + g) * 128, :],
                                            in_=adE[:, g, :])
                res = {"S": None, "msgp": msgp, "h": h_ch}
                chunk_tiles[s][k] = res
                return res
            eL = pBs.tile([128, GPC * 8], fp32, tag=f"eL{s}")
            nc.vector.tensor_tensor(
                out=eL[:].rearrange("p (g h) -> p g h", h=8),
                in0=asr[:].rearrange("p (g h) -> p g h", h=8),
                in1=adE[:, :, 0:8], op=Alu.add)
            eL2 = pBs.tile([128, GPC * 8], fp32, tag=f"eL2{s}")
            nc.vector.scalar_tensor_tensor(out=eL2[:], in0=eL[:], scalar=0.2,
                                           in1=eL[:], op0=Alu.mult, op1=Alu.max)
            nc.scalar.activation(out=msgp[:, :, 128:136],
                                 in_=eL2[:].rearrange("p (g h) -> p g h", h=8),
                                 func=Act.Exp)
            if bstep < 4:
                if s == 0 and k == 0:
                    nc.sync.dma_start(out=z_d[0:128, :], in_=eL2[:, 0:128])
                    nc.sync.dma_start(out=z_d[128:256, :], in_=eL2[:, 128:256])
                res = {"S": None, "msgp": msgp, "h": h_ch}
                chunk_tiles[s][k] = res
                return res
            nc.vector.tensor_tensor(
                out=msgp[:, :, 0:128].rearrange("p g (h f) -> p g h f", f=F),
                in0=h_ch[:].rearrange("p g (h f) -> p g h f", f=F),
                in1=msgp[:, :, 128:136].to_broadcast([128, GPC, 8, F]),
                op=Alu.mult)
            S_ch = pB.tile([128, GPC, BLK], bft, tag=f"S{s}")
            iota_b = AP(iota_s[:].tensor, iota_s[:].offset,
                        [iota_s[:].ap[0], [0, GPC], iota_s[:].ap[1]])
            nc.vector.tensor_tensor(out=S_ch[:], in0=dlt[:].to_broadcast([128, GPC, BLK]),
                                    in1=iota_b, op=Alu.is_equal)
            res = {"S": S_ch, "msgp": msgp}
            chunk_tiles[s][k] = res
            # drop older chunks (keep pool slots bounded by bufs anyway)
            return res

        if run_B and bstep < 5:
            for s in range(2):
                for k in range((st_d[s]["L"]) // CHUNK):
                    emit_chunk(s, k)
        for b in range(NBLK if (run_B and bstep >= 5) else 0):
            ps_blk = psB.tile([128, 136], fp32, tag="blk")
            tot = int(g_prof[b, 0] + g_prof[b, 1])
            done = 0
            for s in range(2):
                for gi in range(int(g_prof[b, s])):
                    gg = int(starts[b, s]) + gi
                    ct = emit_chunk(s, gg // GPC)
                    gl = gg % GPC
                    nc.tensor.matmul(ps_blk[:],
                                     lhsT=ct["S"][:, gl, :],
                                     rhs=ct["msgp"][:, gl, :],
                                     start=(done == 0), stop=(done == tot - 1))
                    done += 1
            # normalize
            rec = pBs.tile([128, 8], fp32, tag="rec")
            nc.vector.reciprocal(out=rec[:], in_=ps_blk[:, 128:136])
            gt = pC.tile([128, 128], fp32, tag="gt")
            nc.vector.tensor_tensor(
                out=gt[:].rearrange("p (h f) -> p h f", f=F),
                in0=ps_blk[:, 0:128].rearrange("p (h f) -> p h f", f=F),
                in1=rec[:].to_broadcast([128, 8, F]), op=Alu.mult)
            if not host["triv_bgat"]:
                nc.vector.tensor_tensor(out=gt[:], in0=gt[:], in1=gl_s["bgat"][:],
                                        op=Alu.add)
            if not run_C:
                nc.sync.dma_start(out=z_d[b * 128:(b + 1) * 128, :], in_=gt[:])
                continue
            # ---- phase C for tile b ----
            xo = pC.tile([128, 128], fp32, tag="xo")
            nc.sync.dma_start(out=xo[:], in_=xown_d[b * 128:(b + 1) * 128, :])
            t1 = pC.tile([128, 128], fp32, tag="t1")
            nc.vector.tensor_tensor(out=t1[:], in0=xo[:], in1=gt[:], op=Alu.add)

            def layer_norm(tin, g_key, b_key, triv, out_dtype, tagp):
                bst = pBs.tile([128, 6], fp32, tag=f"bst{tagp}")
                nc.vector.bn_stats(out=bst[:], in_=tin[:])
                mv = pBs.tile([128, 2], fp32, tag=f"mv{tagp}")
                nc.vector.bn_aggr(out=mv[:], in_=bst[:])
                nc.scalar.activation(out=mv[:, 1:2], in_=mv[:, 1:2],
                                     func=Act.Sqrt, bias=eps_s[:])
                nc.vector.reciprocal(out=mv[:, 1:2], in_=mv[:, 1:2])
                o = pC.tile([128, 128], out_dtype, tag=f"ln{tagp}")
                nc.vector.tensor_scalar(out=o[:], in0=tin[:],
                                        scalar1=mv[:, 0:1], op0=Alu.subtract,
                                        scalar2=mv[:, 1:2], op1=Alu.mult)
                if not triv:
                    nc.vector.tensor_tensor(out=o[:], in0=o[:], in1=gl_s[g_key][:],
                                            op=Alu.mult)
                    nc.vector.tensor_tensor(out=o[:], in0=o[:], in1=gl_s[b_key][:],
                                            op=Alu.add)
                return o

            u = layer_norm(t1, "g1", "b1", host["triv_gb1"], fp32, "1")
            u_bf = pC.tile([128, 128], bft, tag="ubf")
            nc.scalar.activation(out=u_bf[:], in_=u[:], func=Act.Copy)
            uT_ps = psC.tile([128, 128], bft, tag="uT")
            nc.tensor.transpose(uT_ps[:], in_=u_bf[:], identity=I128_s[:])
            uT = pC.tile([128, 128], bft, tag="uTs")
            nc.scalar.activation(out=uT[:], in_=uT_ps[:], func=Act.Copy)
            f1ps = psC.tile([128, 2, 128], fp32, tag="f1")
            for j in range(2):
                nc.tensor.matmul(f1ps[:, j, :], lhsT=W1_s[:, j * 128:(j + 1) * 128],
                                 rhs=uT[:], start=True, stop=True)
            r1 = pC.tile([128, 2, 128], bft, tag="r1")
            for j in range(2):
                nc.scalar.activation(out=r1[:, j, :], in_=f1ps[:, j, :],
                                     func=Act.Relu, bias=b1c_s[:, j:j + 1])
            zps = psC.tile([128, 128], fp32, tag="zp")
            for j in range(2):
                nc.tensor.matmul(zps[:], lhsT=r1[:, j, :], rhs=W2_s[:, j, :],
                                 start=(j == 0), stop=(j == 1))
            t2 = pC.tile([128, 128], fp32, tag="t2")
            nc.vector.tensor_tensor(out=t2[:], in0=u[:], in1=zps[:], op=Alu.add)
            if not host["triv_bff2"]:
                nc.vector.tensor_tensor(out=t2[:], in0=t2[:], in1=gl_s["bff2"][:],
                                        op=Alu.add)
            zt = layer_norm(t2, "g2", "b2", host["triv_gb2"], fp32, "2")
            nc.sync.dma_start(out=z_d[b * 128:(b + 1) * 128, :], in_=zt[:])

        for p in (psC, pC, psB, pBs, pB):
            p.release()
        cpool.release()

    nc.compile()
    return nc


def kernel(**inputs):
    from concourse.bass_utils import run_bass_kernel_spmd

    import os as _os
    host = _build_host_data(inputs)
    nc = _build_program(host, phases=_os.environ.get("GAT_PHASES", "ABC"),
                        bstep=int(_os.environ.get("GAT_BSTEP", "5")))

    in_maps = []
    for c in range(NCORES):
        m = {
            "xT": host["xT"][c],
            "x_own": host["x_own"][c],
            "Wp": host["Wp"], "attS": host["attS"], "iota": host["iota"],
            "I128": host["I128"],
            "W1": host["W1"], "W2": host["W2"], "b1col": host["b1col"],
        }
        if not host["triv_bgat"]:
            m["bgat_r"] = np.tile(host["bias_gat"].reshape(1, -1), (128, 1))
        if not host["triv_bff2"]:
            m["bff2_r"] = np.tile(host["b_ff2"].reshape(1, -1), (128, 1))
        if not host["triv_gb1"]:
            m["g1_r"] = np.tile(host["gamma1"].reshape(1, -1), (128, 1))
            m["b1_r"] = np.tile(host["beta1"].reshape(1, -1), (128, 1))
        if not host["triv_gb2"]:
            m["g2_r"] = np.tile(host["gamma2"].reshape(1, -1), (128, 1))
            m["b2_r"] = np.tile(host["beta2"].reshape(1, -1), (128, 1))
        for s, sname in ((0, "lo"), (1, "hi")):
            sd = host["per_core"][c][s]
            m[f"gidx_{sname}"] = sd["gidx16"]
            m[f"aidx_{sname}"] = sd["aidx16"]
            m[f"dl_{sname}"] = sd["dl"]
        in_maps.append(m)

    import os
    trace = bool(int(os.environ.get("GAT_TRACE", "0")))
    res = run_bass_kernel_spmd(nc, in_maps, core_ids=list(range(NCORES)),
                               trace=trace)
    if trace and res.exec_time_ns:
        print(f"HW exec time: {res.exec_time_ns} ns")
    if bool(int(os.environ.get("GAT_TIME", "0"))):
        try:
            from concourse.timeline_sim import TimelineSim
            ts = TimelineSim(nc)
            dur = ts.simulate()
            print(f"HW exec time: {dur:.0f} ns (cost-model timeline estimate)")
        except Exception as e:
            print("timeline sim failed:", e)

    out = np.zeros((N, D), np.float32)
    for c in range(NCORES):
        lo_n = OWN * c
        hi_n = min(OWN * (c + 1), N)
        out[lo_n:hi_n] = res.results[c]["z"][: hi_n - lo_n]
    return out



# revision 3
# speedup vs baseline: 1.2019x; 1.2019x over previous
"""Trainium2 Bass kernel for a GAT block (GATConv + LN + FFN + LN).

Self-contained: builds per-core shards on the host, compiles one SPMD Bass
program, runs it on 8 NeuronCores via run_bass_kernel_spmd, and reassembles
the full [50000, 128] output.

Per-core scheme (core c of 8, nodes permuted own|zeropad|rest):
  Phase A: for all 50304 (padded) rows compute [h | a_src | a_dst] =
           x @ [W | W@Asrc | W@Adst] on PE; store 512-byte node rows
           [h(128) | a_src(8) | a_dst(8) | onehot_fp8(64 bf16 slots) | pad]
           to core-local DRAM. The onehot block is a constant identity
           pattern (row r holds onehot(r % 128) in fp8) that later serves
           as ready-made scatter-matmul weights.
  Phase B: edges with dst owned by the core (incl self-loops), grouped by
           128-node dst block, split lo/hi on the 32K int16 gather-index
           limit, padded per (block, stream) to 128-edge granules with a
           shared max-over-cores profile so all cores run one program.
           Per 4096-edge chunk: gather#1 512B src rows (h + a_src);
           gather#2 256B dst meta half-rows (a_dst + fp8 onehot = S);
           eL = a_src + a_dst; p = exp(leaky_relu(eL)); pexp = head-expand
           of p on the Act engine; msg = h * pexp (DVE 2x); per granule:
           psum[block] += S_fp8^T-matmul [msg | p].
  Phase C: g = agg/denom; u = LN(x + g); ff = relu(u@W1 + b1)@W2 + b2;
           z = LN(u + ff). rstd via exp(-0.5*ln(var+eps)) keeps every
           activation in one table set (no LoadActFuncSet churn).
"""
import numpy as np
import ml_dtypes

N = 50000
NCORES = 8
OWN = 6272             # nodes per core (49 tiles of 128)
ZPAD = 128             # zero rows after own block (pad-edge target)
NPG = OWN * NCORES     # globally padded node count (50176)
NP2 = NPG + ZPAD       # per-core row count (50304)
BLK = 128
NBLK = OWN // BLK      # 49
GR = 128               # edges per granule
CHUNK = 4096
GPC = CHUNK // GR      # 32
LO_LIM = 1 << 15
H, F, D = 8, 16, 128
ROW = 256              # bf16 cols per node row (512 bytes)
C_AS = 128             # a_src col
C_AD = 136             # a_dst col
C_OH = 144             # onehot (fp8-as-bf16) col, 64 cols
LN_EPS = 1e-5

bf16 = ml_dtypes.bfloat16
f8 = ml_dtypes.float8_e4m3fn


def _wrapc(idx):
    """Per-chunk 16-wrap: [CHUNK] int -> [128, CHUNK//16] int16."""
    w = idx.reshape(CHUNK // 16, 16).T.astype(np.int16)
    return np.tile(w, (8, 1))


def _bfr(x):
    return np.ascontiguousarray(x, dtype=np.float32).astype(bf16)


def _build_host_data(inputs):
    x = np.asarray(inputs["x"], np.float32)
    W = np.asarray(inputs["W_gat"], np.float32)
    att_src = np.asarray(inputs["att_src"], np.float32)
    att_dst = np.asarray(inputs["att_dst"], np.float32)
    ei = np.asarray(inputs["edge_index"])

    src = ei[0].astype(np.int64)
    dst = ei[1].astype(np.int64)
    loops = np.arange(N, dtype=np.int64)
    src = np.concatenate([src, loops])
    dst = np.concatenate([dst, loops])

    # per-core row index of global padded node g:
    #   own -> [0, OWN); zeros -> [OWN, OWN+ZPAD); rest keeps order after
    def inv_row(c, g):
        own = (g >= OWN * c) & (g < OWN * (c + 1))
        r = np.where(own, g - OWN * c,
                     OWN + ZPAD + np.where(g < OWN * c, g, g - OWN))
        return r

    # per (core, block, stream) counts on row-mapped gather indices
    counts = np.zeros((NCORES, NBLK, 2), dtype=np.int64)
    core_edges = []
    for c in range(NCORES):
        m = (dst >= OWN * c) & (dst < min(OWN * (c + 1), N))
        s_g = inv_row(c, src[m])
        d_l = dst[m] - OWN * c
        blk = d_l // BLK
        lo = s_g < LO_LIM
        core_edges.append((s_g, d_l, blk, lo))
        for b in range(NBLK):
            mb = blk == b
            counts[c, b, 0] = np.sum(mb & lo)
            counts[c, b, 1] = np.sum(mb & ~lo)

    g_prof = np.ceil(counts.max(axis=0) / GR).astype(np.int64)   # [NBLK, 2]
    L = [int(g_prof[:, s].sum()) * GR for s in range(2)]
    for s in range(2):
        pad = (-L[s]) % CHUNK
        g_prof[NBLK - 1, s] += pad // GR
        L[s] += pad
    L_LO, L_HI = L

    per_core = []
    for c in range(NCORES):
        s_g, d_l, blk, lo = core_edges[c]
        streams = []
        for sidx in range(2):
            mm = lo if sidx == 0 else ~lo
            Ls = L[sidx]
            gidx = np.zeros(Ls, dtype=np.int64)
            aidx = np.full(Ls, OWN, dtype=np.int64)   # pads -> zero row
            pos = 0
            for b in range(NBLK):
                mb = (blk == b) & mm
                k = int(np.sum(mb))
                cap = int(g_prof[b, sidx]) * GR
                gidx[pos:pos + k] = s_g[mb] - (0 if sidx == 0 else LO_LIM)
                aidx[pos:pos + k] = d_l[mb]
                pos += cap
            # pack [gidx | aidx] wrapped per chunk: [128, nch*512] i16
            nch = Ls // CHUNK
            pk = np.zeros((128, nch * 512), dtype=np.int16)
            for k in range(nch):
                pk[:, k * 512:k * 512 + 256] = _wrapc(gidx[k * CHUNK:(k + 1) * CHUNK])
                pk[:, k * 512 + 256:(k + 1) * 512] = _wrapc(aidx[k * CHUNK:(k + 1) * CHUNK])
            streams.append({"idx": np.ascontiguousarray(pk)})
        per_core.append(streams)

    # weights: Wp = [W | W@Asrc | W@Adst]  -> [128, 144]
    Asrc = np.zeros((D, H), np.float32)
    Adst = np.zeros((D, H), np.float32)
    for h in range(H):
        Asrc[h * F:(h + 1) * F, h] = att_src[h]
        Adst[h * F:(h + 1) * F, h] = att_dst[h]
    Wp = _bfr(np.concatenate([W, W @ Asrc, W @ Adst], axis=1))   # [128, 144]
    I128 = _bfr(np.eye(128, dtype=np.float32))
    # onehot identity as fp8 bytes viewed as bf16: [128, 64]
    oh = np.zeros((128, 128), dtype=f8)
    for i in range(128):
        oh[i, i] = 1.0
    onehotc = np.ascontiguousarray(oh).view(np.uint16).view(bf16)  # [128, 64]

    xp = np.zeros((NP2, D), np.float32)
    xp[:N] = x                         # global padded layout first
    xT_per_core = []
    x_own_per_core = []
    for c in range(NCORES):
        rows = np.zeros((NP2, D), np.float32)
        rows[0:OWN] = xp[OWN * c: OWN * (c + 1)]
        rest = np.concatenate([xp[: OWN * c], xp[OWN * (c + 1): NPG]])
        rows[OWN + ZPAD:] = rest
        xT_per_core.append(np.ascontiguousarray(rows.T.astype(bf16)))
        x_own_per_core.append(np.ascontiguousarray(xp[OWN * c: OWN * (c + 1)]))

    host = {
        "g_prof": g_prof, "L_LO": L_LO, "L_HI": L_HI,
        "per_core": per_core, "xT": xT_per_core, "x_own": x_own_per_core,
        "Wp": Wp, "I128": I128, "onehotc": onehotc,
        "W1": _bfr(np.asarray(inputs["w_ff1"], np.float32)),
        "W2": _bfr(np.asarray(inputs["w_ff2"], np.float32)),
        "b1col": np.ascontiguousarray(
            np.asarray(inputs["b_ff1"], np.float32).reshape(2, 128).T),
    }
    host["bias_gat"] = np.asarray(inputs["bias_gat"], np.float32)
    host["b_ff2"] = np.asarray(inputs["b_ff2"], np.float32)
    for nm in ("gamma1", "beta1", "gamma2", "beta2"):
        host[nm] = np.asarray(inputs[nm], np.float32)
    host["triv_gb1"] = bool(np.all(host["gamma1"] == 1) and np.all(host["beta1"] == 0))
    host["triv_gb2"] = bool(np.all(host["gamma2"] == 1) and np.all(host["beta2"] == 0))
    host["triv_bgat"] = bool(np.all(host["bias_gat"] == 0))
    host["triv_bff2"] = bool(np.all(host["b_ff2"] == 0))
    return host


def _build_program(host):
    import concourse.bacc as bacc
    import concourse.mybir as mybir
    import concourse.tile as tile

    fp32 = mybir.dt.float32
    bft = mybir.dt.bfloat16
    i16 = mybir.dt.int16
    fp8 = mybir.dt.float8e4
    Alu = mybir.AluOpType
    Act = mybir.ActivationFunctionType

    g_prof = host["g_prof"]
    L_LO, L_HI = host["L_LO"], host["L_HI"]

    nc = bacc.Bacc("TRN2")

    xT_d = nc.dram_tensor("xT", [128, NP2], bft, kind="ExternalInput")
    xown_d = nc.dram_tensor("x_own", [OWN, D], fp32, kind="ExternalInput")
    Wp_d = nc.dram_tensor("Wp", [128, 144], bft, kind="ExternalInput")
    I128_d = nc.dram_tensor("I128", [128, 128], bft, kind="ExternalInput")
    oh_d = nc.dram_tensor("onehotc", [128, 64], bft, kind="ExternalInput")
    W1_d = nc.dram_tensor("W1", [128, 256], bft, kind="ExternalInput")
    W2_d = nc.dram_tensor("W2", [256, 128], bft, kind="ExternalInput")
    b1c_d = nc.dram_tensor("b1col", [128, 2], fp32, kind="ExternalInput")
    gl_d = {}
    if not host["triv_bgat"]:
        gl_d["bgat"] = nc.dram_tensor("bgat_r", [128, 128], fp32, kind="ExternalInput")
    if not host["triv_bff2"]:
        gl_d["bff2"] = nc.dram_tensor("bff2_r", [128, 128], fp32, kind="ExternalInput")
    if not host["triv_gb1"]:
        gl_d["g1"] = nc.dram_tensor("g1_r", [128, 128], fp32, kind="ExternalInput")
        gl_d["b1"] = nc.dram_tensor("b1_r", [128, 128], fp32, kind="ExternalInput")
    if not host["triv_gb2"]:
        gl_d["g2"] = nc.dram_tensor("g2_r", [128, 128], fp32, kind="ExternalInput")
        gl_d["b2"] = nc.dram_tensor("b2_r", [128, 128], fp32, kind="ExternalInput")

    st_d = []
    for sname, Ls in (("lo", L_LO), ("hi", L_HI)):
        st_d.append({
            "idx": nc.dram_tensor(f"idx_{sname}", [128, (Ls // CHUNK) * 512], i16,
                                  kind="ExternalInput"),
            "L": Ls,
        })

    h_d = nc.dram_tensor("h_scratch", [NP2, ROW], bft, kind="Internal")
    z_d = nc.dram_tensor("z", [OWN, D], fp32, kind="ExternalOutput")

    NT2 = NP2 // 128                  # 393 node tiles
    PADT = OWN // 128                 # tile 49 == the zero-pad block
    GT = 3                            # node tiles per psum bank
    SW = 12                           # node tiles per stage flush / x DMA

    with tile.TileContext(nc) as tc:
        # ================= consts =================
        cpool = tc.alloc_tile_pool(name="consts", bufs=1)
        Wp_s = cpool.tile([128, 144], bft)
        nc.sync.dma_start(out=Wp_s[:], in_=Wp_d[:])
        I128_s = cpool.tile([128, 128], bft)
        nc.sync.dma_start(out=I128_s[:], in_=I128_d[:])
        W1_s = cpool.tile([128, 256], bft)
        nc.sync.dma_start(out=W1_s[:], in_=W1_d[:])
        W2_s = cpool.tile([256 // 2, 2, 128], bft)
        nc.sync.dma_start(out=W2_s[:],
                          in_=W2_d[:].rearrange("(k h) f -> h k f", k=2))
        b1c_s = cpool.tile([128, 2], fp32)
        nc.sync.dma_start(out=b1c_s[:], in_=b1c_d[:])
        gl_s = {}
        for k, dref in gl_d.items():
            gl_s[k] = cpool.tile([128, 128], fp32, tag=f"gl_{k}")
            nc.sync.dma_start(out=gl_s[k][:], in_=dref[:])
        eps_s = cpool.tile([128, 1], fp32)
        nc.vector.memset(eps_s[:], LN_EPS)
        zt64 = cpool.tile([128, 64], bft)
        nc.vector.memset(zt64[:], 0.0)
        # two fixed stage buffers with persistent onehot-const region
        stg = []
        for i in range(2):
            s = cpool.tile([128, SW, ROW], bft, tag=f"stage{i}")
            nc.vector.memset(s[:], 0.0)
            nc.sync.dma_start(
                out=s[:, :, C_OH:C_OH + 64],
                in_=oh_d[:].rearrange("p (o f) -> p o f", o=1).to_broadcast(
                    [128, SW, 64]))
            stg.append(s)

        # ================= phase A =================
        with tc.tile_pool(name="pA", bufs=4) as pA, \
             tc.tile_pool(name="psA", bufs=4, space="PSUM") as psA:
            xt = None
            for tg in range((NT2 + GT - 1) // GT):
                t0 = tg * GT
                ntl = min(GT, NT2 - t0)
                if t0 % SW == 0:
                    nxb = min(SW, NT2 - t0)
                    xt = pA.tile([128, SW * 128], bft, tag="xt")
                    nc.sync.dma_start(out=xt[:, :nxb * 128],
                                      in_=xT_d[:, t0 * 128:(t0 + nxb) * 128])
                ps = psA.tile([128, GT, 144], fp32, tag="psA")
                for j in range(ntl):
                    jo = (t0 % SW) + j
                    nc.tensor.matmul(ps[:, j, :],
                                     lhsT=xt[:, jo * 128:(jo + 1) * 128],
                                     rhs=Wp_s[:], start=True, stop=True)
                sb = stg[(t0 // SW) % 2]
                j0 = t0 % SW
                if tg % 2 == 0:
                    nc.scalar.activation(out=sb[:, j0:j0 + ntl, 0:144],
                                         in_=ps[:, :ntl, :], func=Act.Copy)
                else:
                    nc.vector.tensor_copy(out=sb[:, j0:j0 + ntl, 0:144],
                                          in_=ps[:, :ntl, :])
                if j0 + ntl == SW or t0 + ntl == NT2:
                    nf = j0 + ntl
                    r0 = (t0 + ntl - nf) * 128
                    nc.sync.dma_start(
                        out=h_d[r0:r0 + nf * 128, :].rearrange(
                            "(j n) d -> n j d", j=nf),
                        in_=sb[:, :nf, :])
            # zero the onehot region of the zero-pad block rows
            nc.sync.dma_start(out=h_d[OWN:OWN + ZPAD, C_OH:C_OH + 64],
                              in_=zt64[:])

        tc.strict_bb_all_engine_barrier()

        # ================= phases B + C =================
        h_lo = h_d[0:LO_LIM, :]
        h_hi = h_d[LO_LIM:NP2, :]
        meta_tab = h_d[:, 128:256]     # [NP2, 128] at 512B pitch
        starts = np.zeros((NBLK, 2), dtype=np.int64)
        for s in range(2):
            starts[1:, s] = np.cumsum(g_prof[:-1, s])

        pB = tc.alloc_tile_pool(name="pB", bufs=2)
        pBs = tc.alloc_tile_pool(name="pBsmall", bufs=4)
        psB = tc.alloc_tile_pool(name="psB", bufs=3, space="PSUM")
        pC = tc.alloc_tile_pool(name="pC", bufs=2)
        psC = tc.alloc_tile_pool(name="psC", bufs=1, space="PSUM")

        chunk_tiles = [{}, {}]

        def emit_chunk(s, k):
            if k in chunk_tiles[s]:
                return chunk_tiles[s][k]
            sd = st_d[s]
            idx = pBs.tile([128, 512], i16, tag=f"idx{s}")
            nc.sync.dma_start(out=idx[:],
                              in_=sd["idx"][:, k * 512:(k + 1) * 512])
            hrow = pB.tile([128, GPC, ROW], bft, tag=f"h{s}")
            nc.gpsimd.dma_gather(hrow[:], h_lo if s == 0 else h_hi,
                                 idx[:, 0:256], CHUNK, CHUNK, ROW,
                                 single_packet=False)
            meta = pB.tile([128, GPC, 128], bft, tag=f"m{s}")
            nc.gpsimd.dma_gather(meta[:], meta_tab, idx[:, 256:512],
                                 CHUNK, CHUNK, 128, elem_step=ROW,
                                 single_packet=False)
            # eL = a_src[src] + a_dst[dst]
            eL = pBs.tile([128, GPC, 8], bft, tag=f"eL{s}")
            nc.vector.tensor_tensor(out=eL[:], in0=hrow[:, :, C_AS:C_AS + 8],
                                    in1=meta[:, :, 8:16], op=Alu.add)
            eL2 = pBs.tile([128, GPC, 8], bft, tag=f"eL2{s}")
            nc.vector.scalar_tensor_tensor(out=eL2[:], in0=eL[:], scalar=0.2,
                                           in1=eL[:], op0=Alu.mult, op1=Alu.max)
            msgp = pB.tile([128, GPC, 136], bft, tag=f"msgp{s}")
            nc.scalar.activation(out=msgp[:, :, 128:136], in_=eL2[:],
                                 func=Act.Exp)
            pexp = pB.tile([128, GPC, 128], bft, tag=f"px{s}")
            nc.scalar.activation(
                out=pexp[:].rearrange("p g (h f) -> p g h f", f=F),
                in_=eL2[:].to_broadcast([128, GPC, 8, F]), func=Act.Exp)
            nc.vector.tensor_tensor(out=msgp[:, :, 0:128],
                                    in0=hrow[:, :, 0:128], in1=pexp[:],
                                    op=Alu.mult)
            res = {"S": meta, "msgp": msgp}
            chunk_tiles[s][k] = res
            return res

        for b in range(NBLK):
            ps_blk = psB.tile([128, 136], fp32, tag="blk")
            tot = int(g_prof[b, 0] + g_prof[b, 1])
            done = 0
            for s in range(2):
                for gi in range(int(g_prof[b, s])):
                    gg = int(starts[b, s]) + gi
                    ct = emit_chunk(s, gg // GPC)
                    gl = gg % GPC
                    nc.tensor.matmul(
                        ps_blk[:],
                        lhsT=ct["S"][:, gl, 16:80].bitcast(fp8),
                        rhs=ct["msgp"][:, gl, :],
                        start=(done == 0), stop=(done == tot - 1))
                    done += 1
            rec = pBs.tile([128, 8], fp32, tag="rec")
            nc.vector.reciprocal(out=rec[:], in_=ps_blk[:, 128:136])
            gt = pC.tile([128, 128], fp32, tag="gt")
            nc.vector.tensor_tensor(
                out=gt[:].rearrange("p (h f) -> p h f", f=F),
                in0=ps_blk[:, 0:128].rearrange("p (h f) -> p h f", f=F),
                in1=rec[:].to_broadcast([128, 8, F]), op=Alu.mult)
            if not host["triv_bgat"]:
                nc.vector.tensor_tensor(out=gt[:], in0=gt[:], in1=gl_s["bgat"][:],
                                        op=Alu.add)
            # ---- phase C for tile b ----
            xo = pC.tile([128, 128], fp32, tag="xo")
            nc.sync.dma_start(out=xo[:], in_=xown_d[b * 128:(b + 1) * 128, :])
            t1 = pC.tile([128, 128], fp32, tag="t1")
            nc.vector.tensor_tensor(out=t1[:], in0=xo[:], in1=gt[:], op=Alu.add)

            def layer_norm(tin, g_key, b_key, triv, tagp):
                bst = pBs.tile([128, 6], fp32, tag=f"bst{tagp}")
                nc.vector.bn_stats(out=bst[:], in_=tin[:])
                mv = pBs.tile([128, 2], fp32, tag=f"mv{tagp}")
                nc.vector.bn_aggr(out=mv[:], in_=bst[:])
                # rstd = exp(-0.5 * ln(var + eps)); stays in the exp/ln set
                nc.scalar.activation(out=mv[:, 1:2], in_=mv[:, 1:2],
                                     func=Act.Ln, bias=eps_s[:])
                nc.scalar.activation(out=mv[:, 1:2], in_=mv[:, 1:2],
                                     func=Act.Exp, scale=-0.5)
                o = pC.tile([128, 128], fp32, tag=f"ln{tagp}")
                nc.vector.tensor_scalar(out=o[:], in0=tin[:],
                                        scalar1=mv[:, 0:1], op0=Alu.subtract,
                                        scalar2=mv[:, 1:2], op1=Alu.mult)
                if not triv:
                    nc.vector.tensor_tensor(out=o[:], in0=o[:], in1=gl_s[g_key][:],
                                            op=Alu.mult)
                    nc.vector.tensor_tensor(out=o[:], in0=o[:], in1=gl_s[b_key][:],
                                            op=Alu.add)
                return o

            u = layer_norm(t1, "g1", "b1", host["triv_gb1"], "1")
            u_bf = pC.tile([128, 128], bft, tag="ubf")
            nc.scalar.activation(out=u_bf[:], in_=u[:], func=Act.Copy)
            uT_ps = psC.tile([128, 128], bft, tag="uT")
            nc.tensor.transpose(uT_ps[:], in_=u_bf[:], identity=I128_s[:])
            uT = pC.tile([128, 128], bft, tag="uTs")
            nc.scalar.activation(out=uT[:], in_=uT_ps[:], func=Act.Copy)
            f1ps = psC.tile([128, 2, 128], fp32, tag="f1")
            for j in range(2):
                nc.tensor.matmul(f1ps[:, j, :], lhsT=W1_s[:, j * 128:(j + 1) * 128],
                                 rhs=uT[:], start=True, stop=True)
            r1 = pC.tile([128, 2, 128], bft, tag="r1")
            for j in range(2):
                nc.scalar.activation(out=r1[:, j, :], in_=f1ps[:, j, :],
                                     func=Act.Relu, bias=b1c_s[:, j:j + 1])
            zps = psC.tile([128, 128], fp32, tag="zp")
            for j in range(2):
                nc.tensor.matmul(zps[:], lhsT=r1[:, j, :], rhs=W2_s[:, j, :],
                                 start=(j == 0), stop=(j == 1))
            t2 = pC.tile([128, 128], fp32, tag="t2")
            nc.vector.tensor_tensor(out=t2[:], in0=u[:], in1=zps[:], op=Alu.add)
            if not host["triv_bff2"]:
                nc.vector.tensor_tensor(out=t2[:], in0=t2[:], in1=gl_s["bff2"][:],
                                        op=Alu.add)
            zt = layer_norm(t2, "g2", "b2", host["triv_gb2"], "2")
            nc.sync.dma_start(out=z_d[b * 128:(b + 1) * 128, :], in_=zt[:])

        for p in (psC, pC, psB, pBs, pB):
            p.release()
        cpool.release()

    nc.compile()
    return nc


def kernel(**inputs):
    from concourse.bass_utils import run_bass_kernel_spmd
    import os

    host = _build_host_data(inputs)
    nc = _build_program(host)

    in_maps = []
    for c in range(NCORES):
        m = {
            "xT": host["xT"][c],
            "x_own": host["x_own"][c],
            "Wp": host["Wp"], "I128": host["I128"], "onehotc": host["onehotc"],
            "W1": host["W1"], "W2": host["W2"], "b1col": host["b1col"],
        }
        if not host["triv_bgat"]:
            m["bgat_r"] = np.tile(host["bias_gat"].reshape(1, -1), (128, 1))
        if not host["triv_bff2"]:
            m["bff2_r"] = np.tile(host["b_ff2"].reshape(1, -1), (128, 1))
        if not host["triv_gb1"]:
            m["g1_r"] = np.tile(host["gamma1"].reshape(1, -1), (128, 1))
            m["b1_r"] = np.tile(host["beta1"].reshape(1, -1), (128, 1))
        if not host["triv_gb2"]:
            m["g2_r"] = np.tile(host["gamma2"].reshape(1, -1), (128, 1))
            m["b2_r"] = np.tile(host["beta2"].reshape(1, -1), (128, 1))
        for s, sname in ((0, "lo"), (1, "hi")):
            m[f"idx_{sname}"] = host["per_core"][c][s]["idx"]
        in_maps.append(m)

    trace = bool(int(os.environ.get("GAT_TRACE", "0")))
    res = run_bass_kernel_spmd(nc, in_maps, core_ids=list(range(NCORES)),
                               trace=trace)
    if trace and res.exec_time_ns:
        print(f"HW exec time: {res.exec_time_ns} ns")
    if bool(int(os.environ.get("GAT_TIME", "0"))):
        try:
            from concourse.timeline_sim import TimelineSim
            ts = TimelineSim(nc)
            dur = ts.simulate()
            print(f"HW exec time: {dur:.0f} ns (cost-model timeline estimate)")
        except Exception as e:
            print("timeline sim failed:", e)

    out = np.zeros((N, D), np.float32)
    for c in range(NCORES):
        lo_n = OWN * c
        hi_n = min(OWN * (c + 1), N)
        out[lo_n:hi_n] = res.results[c]["z"][: hi_n - lo_n]
    return out


# revision 11
# speedup vs baseline: 1.2829x; 1.0674x over previous
"""Trainium2 Bass kernel for a GAT block (GATConv + LN + FFN + LN).

Self-contained: builds per-core shards on the host, compiles one SPMD Bass
program, runs it on 8 NeuronCores via run_bass_kernel_spmd, and reassembles
the full [50000, 128] output.

Per-core scheme (core c of 8, nodes permuted own|zeropad|rest):
  Phase A: for all 50304 (padded) rows compute [h | a_src | a_dst] =
           x @ [W | W@Asrc | W@Adst] on PE; store 512-byte node rows
           [h(128) | a_src(8) | a_dst(8) | onehot_fp8(64 bf16 slots) | pad]
           to core-local DRAM. The onehot block is a constant identity
           pattern (row r holds onehot(r % 128) in fp8) that later serves
           as ready-made scatter-matmul weights.
  Phase B: edges with dst owned by the core (incl self-loops), grouped by
           128-node dst block, split lo/hi on the 32K int16 gather-index
           limit, padded per (block, stream) to 128-edge granules with a
           shared max-over-cores profile so all cores run one program.
           Per 4096-edge chunk: gather#1 512B src rows (h + a_src);
           gather#2 256B dst meta half-rows (a_dst + fp8 onehot = S);
           eL = a_src + a_dst; p = exp(leaky_relu(eL)); pexp = head-expand
           of p on the Act engine; msg = h * pexp (DVE 2x); per granule:
           psum[block] += S_fp8^T-matmul [msg | p].
  Phase C: g = agg/denom; u = LN(x + g); ff = relu(u@W1 + b1)@W2 + b2;
           z = LN(u + ff). rstd via exp(-0.5*ln(var+eps)) keeps every
           activation in one table set (no LoadActFuncSet churn).
"""
import numpy as np
import ml_dtypes

N = 50000
NCORES = 8
OWN = 6272             # nodes per core (49 tiles of 128)
ZPAD = 128             # zero rows after own block (pad-edge target)
NPG = OWN * NCORES     # globally padded node count (50176)
NP2 = NPG + ZPAD       # per-core row count (50304)
BLK = 128
NBLK = OWN // BLK      # 49
GR = 128               # edges per granule
CHUNK = 4096
GPC = CHUNK // GR      # 32
LO_LIM = 1 << 15
H, F, D = 8, 16, 128
ROW = 256              # bf16 cols per node row (512 bytes)
C_AS = 128             # a_src col
C_AD = 136             # a_dst col
C_OH = 144             # onehot (fp8-as-bf16) col, 64 cols
LN_EPS = 1e-5

bf16 = ml_dtypes.bfloat16
f8 = ml_dtypes.float8_e4m3fn


def _wrapc(idx):
    """Per-chunk 16-wrap: [CHUNK] int -> [128, CHUNK//16] int16."""
    w = idx.reshape(CHUNK // 16, 16).T.astype(np.int16)
    return np.tile(w, (8, 1))


def _bfr(x):
    return np.ascontiguousarray(x, dtype=np.float32).astype(bf16)


def _build_host_data(inputs):
    x = np.asarray(inputs["x"], np.float32)
    W = np.asarray(inputs["W_gat"], np.float32)
    att_src = np.asarray(inputs["att_src"], np.float32)
    att_dst = np.asarray(inputs["att_dst"], np.float32)
    ei = np.asarray(inputs["edge_index"])

    src = ei[0].astype(np.int64)
    dst = ei[1].astype(np.int64)
    loops = np.arange(N, dtype=np.int64)
    src = np.concatenate([src, loops])
    dst = np.concatenate([dst, loops])

    # per-core row index of global padded node g:
    #   own -> [0, OWN); zeros -> [OWN, OWN+ZPAD); rest keeps order after
    def inv_row(c, g):
        own = (g >= OWN * c) & (g < OWN * (c + 1))
        r = np.where(own, g - OWN * c,
                     OWN + ZPAD + np.where(g < OWN * c, g, g - OWN))
        return r

    # per (core, block, stream) counts on row-mapped gather indices
    counts = np.zeros((NCORES, NBLK, 2), dtype=np.int64)
    core_edges = []
    for c in range(NCORES):
        m = (dst >= OWN * c) & (dst < min(OWN * (c + 1), N))
        s_g = inv_row(c, src[m])
        d_l = dst[m] - OWN * c
        blk = d_l // BLK
        lo = s_g < LO_LIM
        core_edges.append((s_g, d_l, blk, lo))
        for b in range(NBLK):
            mb = blk == b
            counts[c, b, 0] = np.sum(mb & lo)
            counts[c, b, 1] = np.sum(mb & ~lo)

    g_prof = np.ceil(counts.max(axis=0) / GR).astype(np.int64)   # [NBLK, 2]
    L = [int(g_prof[:, s].sum()) * GR for s in range(2)]
    for s in range(2):
        pad = (-L[s]) % CHUNK
        g_prof[NBLK - 1, s] += pad // GR
        L[s] += pad
    L_LO, L_HI = L

    per_core = []
    for c in range(NCORES):
        s_g, d_l, blk, lo = core_edges[c]
        streams = []
        for sidx in range(2):
            mm = lo if sidx == 0 else ~lo
            Ls = L[sidx]
            gidx = np.zeros(Ls, dtype=np.int64)
            aidx = np.full(Ls, OWN, dtype=np.int64)   # pads -> zero row
            pos = 0
            for b in range(NBLK):
                mb = (blk == b) & mm
                k = int(np.sum(mb))
                cap = int(g_prof[b, sidx]) * GR
                gidx[pos:pos + k] = s_g[mb] - (0 if sidx == 0 else LO_LIM)
                aidx[pos:pos + k] = d_l[mb]
                pos += cap
            # pack [gidx | aidx] wrapped per chunk: [128, nch*512] i16
            nch = Ls // CHUNK
            pk = np.zeros((128, nch * 512), dtype=np.int16)
            for k in range(nch):
                pk[:, k * 512:k * 512 + 256] = _wrapc(gidx[k * CHUNK:(k + 1) * CHUNK])
                pk[:, k * 512 + 256:(k + 1) * 512] = _wrapc(aidx[k * CHUNK:(k + 1) * CHUNK])
            streams.append({"idx": np.ascontiguousarray(pk)})
        per_core.append(streams)

    # weights: Wp = [W | W@Asrc | W@Adst]  -> [128, 144]
    Asrc = np.zeros((D, H), np.float32)
    Adst = np.zeros((D, H), np.float32)
    for h in range(H):
        Asrc[h * F:(h + 1) * F, h] = att_src[h]
        Adst[h * F:(h + 1) * F, h] = att_dst[h]
    Wp = _bfr(np.concatenate([W, W @ Asrc, W @ Adst], axis=1))   # [128, 144]
    I128 = _bfr(np.eye(128, dtype=np.float32))
    # onehot identity as fp8 bytes viewed as bf16: [128, 64]
    oh = np.zeros((128, 128), dtype=f8)
    for i in range(128):
        oh[i, i] = 1.0
    onehotc = np.ascontiguousarray(oh).view(np.uint16).view(bf16)  # [128, 64]

    xp = np.zeros((NP2, D), np.float32)
    xp[:N] = x                         # global padded layout first
    xT_per_core = []
    x_own_per_core = []
    for c in range(NCORES):
        rows = np.zeros((NP2, D), np.float32)
        rows[0:OWN] = xp[OWN * c: OWN * (c + 1)]
        rest = np.concatenate([xp[: OWN * c], xp[OWN * (c + 1): NPG]])
        rows[OWN + ZPAD:] = rest
        xT_per_core.append(np.ascontiguousarray(rows.T.astype(bf16)))
        x_own_per_core.append(np.ascontiguousarray(xp[OWN * c: OWN * (c + 1)]))

    host = {
        "g_prof": g_prof, "L_LO": L_LO, "L_HI": L_HI,
        "per_core": per_core, "xT": xT_per_core, "x_own": x_own_per_core,
        "Wp": Wp, "I128": I128, "onehotc": onehotc,
        "W1": _bfr(np.asarray(inputs["w_ff1"], np.float32)),
        "W2": _bfr(np.asarray(inputs["w_ff2"], np.float32)),
        "b1col": np.ascontiguousarray(
            np.asarray(inputs["b_ff1"], np.float32).reshape(2, 128).T),
    }
    host["bias_gat"] = np.asarray(inputs["bias_gat"], np.float32)
    host["b_ff2"] = np.asarray(inputs["b_ff2"], np.float32)
    for nm in ("gamma1", "beta1", "gamma2", "beta2"):
        host[nm] = np.asarray(inputs[nm], np.float32)
    host["triv_gb1"] = bool(np.all(host["gamma1"] == 1) and np.all(host["beta1"] == 0))
    host["triv_gb2"] = bool(np.all(host["gamma2"] == 1) and np.all(host["beta2"] == 0))
    host["triv_bgat"] = bool(np.all(host["bias_gat"] == 0))
    host["triv_bff2"] = bool(np.all(host["b_ff2"] == 0))
    return host


def _build_program(host):
    import concourse.bacc as bacc
    import concourse.mybir as mybir
    import concourse.tile as tile

    fp32 = mybir.dt.float32
    bft = mybir.dt.bfloat16
    i16 = mybir.dt.int16
    fp8 = mybir.dt.float8e4
    Alu = mybir.AluOpType
    Act = mybir.ActivationFunctionType

    g_prof = host["g_prof"]
    L_LO, L_HI = host["L_LO"], host["L_HI"]

    nc = bacc.Bacc("TRN2")

    # Pre-place one activation-table load that covers every func we use, so
    # the compile-time fixpoint pass never needs to thrash between sets.
    from concourse.hw_specs import get_activation_tables
    _tabs = list(get_activation_tables(nc.m.arch).items())
    _need = {Act.Exp, Act.Ln, Act.Copy, Act.Relu}
    _set_id = next(i for i, (_n, fns) in enumerate(_tabs) if _need <= fns)

    xT_d = nc.dram_tensor("xT", [128, NP2], bft, kind="ExternalInput")
    xown_d = nc.dram_tensor("x_own", [OWN, D], fp32, kind="ExternalInput")
    Wp_d = nc.dram_tensor("Wp", [128, 144], bft, kind="ExternalInput")
    I128_d = nc.dram_tensor("I128", [128, 128], bft, kind="ExternalInput")
    oh_d = nc.dram_tensor("onehotc", [128, 64], bft, kind="ExternalInput")
    W1_d = nc.dram_tensor("W1", [128, 256], bft, kind="ExternalInput")
    W2_d = nc.dram_tensor("W2", [256, 128], bft, kind="ExternalInput")
    b1c_d = nc.dram_tensor("b1col", [128, 2], fp32, kind="ExternalInput")
    gl_d = {}
    if not host["triv_bgat"]:
        gl_d["bgat"] = nc.dram_tensor("bgat_r", [128, 128], fp32, kind="ExternalInput")
    if not host["triv_bff2"]:
        gl_d["bff2"] = nc.dram_tensor("bff2_r", [128, 128], fp32, kind="ExternalInput")
    if not host["triv_gb1"]:
        gl_d["g1"] = nc.dram_tensor("g1_r", [128, 128], fp32, kind="ExternalInput")
        gl_d["b1"] = nc.dram_tensor("b1_r", [128, 128], fp32, kind="ExternalInput")
    if not host["triv_gb2"]:
        gl_d["g2"] = nc.dram_tensor("g2_r", [128, 128], fp32, kind="ExternalInput")
        gl_d["b2"] = nc.dram_tensor("b2_r", [128, 128], fp32, kind="ExternalInput")

    st_d = []
    for sname, Ls in (("lo", L_LO), ("hi", L_HI)):
        st_d.append({
            "idx": nc.dram_tensor(f"idx_{sname}", [128, (Ls // CHUNK) * 512], i16,
                                  kind="ExternalInput"),
            "L": Ls,
        })

    h_d = nc.dram_tensor("h_scratch", [NP2, ROW], bft, kind="Internal")
    z_d = nc.dram_tensor("z", [OWN, D], fp32, kind="ExternalOutput")

    NT2 = NP2 // 128                  # 393 node tiles
    PADT = OWN // 128                 # tile 49 == the zero-pad block
    GT = 3                            # node tiles per psum bank
    SW = 12                           # node tiles per stage flush / x DMA

    with tile.TileContext(nc) as tc:
        nc.scalar.add_instruction(mybir.InstLoadActFuncSet(
            name=nc.get_next_instruction_name(), ins=[], outs=[],
            act_func_set_id=_set_id))
        # ================= consts =================
        cpool = tc.alloc_tile_pool(name="consts", bufs=1)
        Wp_s = cpool.tile([128, 144], bft)
        nc.sync.dma_start(out=Wp_s[:], in_=Wp_d[:])
        I128_s = cpool.tile([128, 128], bft)
        nc.sync.dma_start(out=I128_s[:], in_=I128_d[:])
        W1_s = cpool.tile([128, 256], bft)
        nc.sync.dma_start(out=W1_s[:], in_=W1_d[:])
        W2_s = cpool.tile([256 // 2, 2, 128], bft)
        nc.sync.dma_start(out=W2_s[:],
                          in_=W2_d[:].rearrange("(k h) f -> h k f", k=2))
        b1c_s = cpool.tile([128, 2], fp32)
        nc.sync.dma_start(out=b1c_s[:], in_=b1c_d[:])
        gl_s = {}
        for k, dref in gl_d.items():
            gl_s[k] = cpool.tile([128, 128], fp32, tag=f"gl_{k}")
            nc.sync.dma_start(out=gl_s[k][:], in_=dref[:])
        eps_s = cpool.tile([128, 1], fp32)
        nc.vector.memset(eps_s[:], LN_EPS)
        zt64 = cpool.tile([128, 64], bft)
        nc.vector.memset(zt64[:], 0.0)
        # two fixed stage buffers with persistent onehot-const region
        stgpool = tc.alloc_tile_pool(name="stg", bufs=1)
        stg = []
        for i in range(2):
            s = stgpool.tile([128, SW, ROW], bft, tag=f"stage{i}")
            nc.vector.memset(s[:], 0.0)
            nc.sync.dma_start(
                out=s[:, :, C_OH:C_OH + 64],
                in_=oh_d[:].rearrange("p (o f) -> p o f", o=1).to_broadcast(
                    [128, SW, 64]))
            stg.append(s)

        # ================= phase A =================
        with tc.tile_pool(name="pA", bufs=4) as pA, \
             tc.tile_pool(name="psA", bufs=4, space="PSUM") as psA:
            xt = None
            for tg in range((NT2 + GT - 1) // GT):
                t0 = tg * GT
                ntl = min(GT, NT2 - t0)
                if t0 % SW == 0:
                    nxb = min(SW, NT2 - t0)
                    xt = pA.tile([128, SW * 128], bft, tag="xt")
                    nc.sync.dma_start(out=xt[:, :nxb * 128],
                                      in_=xT_d[:, t0 * 128:(t0 + nxb) * 128])
                ps = psA.tile([128, GT, 144], fp32, tag="psA")
                for j in range(ntl):
                    jo = (t0 % SW) + j
                    nc.tensor.matmul(ps[:, j, :],
                                     lhsT=xt[:, jo * 128:(jo + 1) * 128],
                                     rhs=Wp_s[:], start=True, stop=True)
                sb = stg[(t0 // SW) % 2]
                j0 = t0 % SW
                if tg % 2 == 0:
                    nc.scalar.activation(out=sb[:, j0:j0 + ntl, 0:144],
                                         in_=ps[:, :ntl, :], func=Act.Copy)
                else:
                    nc.vector.tensor_copy(out=sb[:, j0:j0 + ntl, 0:144],
                                          in_=ps[:, :ntl, :])
                if j0 + ntl == SW or t0 + ntl == NT2:
                    nf = j0 + ntl
                    r0 = (t0 + ntl - nf) * 128
                    nc.sync.dma_start(
                        out=h_d[r0:r0 + nf * 128, :].rearrange(
                            "(j n) d -> n j d", j=nf),
                        in_=sb[:, :nf, :])
            # zero the onehot region of the zero-pad block rows
            nc.sync.dma_start(out=h_d[OWN:OWN + ZPAD, C_OH:C_OH + 64],
                              in_=zt64[:])

        tc.strict_bb_all_engine_barrier()
        stgpool.release()

        # ================= phases B + C =================
        h_lo = h_d[0:LO_LIM, :]
        h_hi = h_d[LO_LIM:NP2, :]
        meta_tab = h_d[:, 128:256]     # [NP2, 128] at 512B pitch
        starts = np.zeros((NBLK, 2), dtype=np.int64)
        for s in range(2):
            starts[1:, s] = np.cumsum(g_prof[:-1, s])

        pB = tc.alloc_tile_pool(name="pB", bufs=4)
        pBs = tc.alloc_tile_pool(name="pBsmall", bufs=6)
        psB = tc.alloc_tile_pool(name="psB", bufs=4, space="PSUM")
        pC = tc.alloc_tile_pool(name="pC", bufs=2)
        psC = tc.alloc_tile_pool(name="psC", bufs=1, space="PSUM")

        chunk_tiles = [{}, {}]

        def emit_chunk(s, k):
            if k in chunk_tiles[s]:
                return chunk_tiles[s][k]
            sd = st_d[s]
            idx = pBs.tile([128, 512], i16, tag="idx")
            nc.scalar.dma_start(out=idx[:],
                                in_=sd["idx"][:, k * 512:(k + 1) * 512])
            hrow = pB.tile([128, GPC, ROW], bft, tag="h")
            nc.gpsimd.dma_gather(hrow[:], h_lo if s == 0 else h_hi,
                                 idx[:, 0:256], CHUNK, CHUNK, ROW,
                                 single_packet=False)
            meta = pB.tile([128, GPC, 128], bft, tag="m")
            nc.gpsimd.dma_gather(meta[:], meta_tab, idx[:, 256:512],
                                 CHUNK, CHUNK, 128, elem_step=ROW,
                                 single_packet=False)
            # eL = a_src[src] + a_dst[dst]
            eL = pBs.tile([128, GPC, 8], bft, tag="eL")
            nc.vector.tensor_tensor(out=eL[:], in0=hrow[:, :, C_AS:C_AS + 8],
                                    in1=meta[:, :, 8:16], op=Alu.add)
            eL2 = pBs.tile([128, GPC, 8], bft, tag="eL2")
            nc.vector.scalar_tensor_tensor(out=eL2[:], in0=eL[:], scalar=0.2,
                                           in1=eL[:], op0=Alu.mult, op1=Alu.max)
            # p over the spent a_src cols; msg in place over h
            nc.scalar.activation(out=hrow[:, :, 128:136], in_=eL2[:],
                                 func=Act.Exp)
            pexp = pB.tile([128, GPC, 128], bft, tag="px")
            nc.scalar.activation(
                out=pexp[:].rearrange("p g (h f) -> p g h f", f=F),
                in_=eL2[:].to_broadcast([128, GPC, 8, F]), func=Act.Exp)
            nc.vector.tensor_tensor(out=hrow[:, :, 0:128],
                                    in0=hrow[:, :, 0:128], in1=pexp[:],
                                    op=Alu.mult)
            res = {"S": meta, "msgp": hrow}
            chunk_tiles[s][k] = res
            return res

        for b in range(NBLK):
            ps_blk = psB.tile([128, 136], fp32, tag="blk")
            tot = int(g_prof[b, 0] + g_prof[b, 1])
            done = 0
            for s in range(2):
                for gi in range(int(g_prof[b, s])):
                    gg = int(starts[b, s]) + gi
                    ct = emit_chunk(s, gg // GPC)
                    gl = gg % GPC
                    nc.tensor.matmul(
                        ps_blk[:],
                        lhsT=ct["S"][:, gl, 16:80].bitcast(fp8),
                        rhs=ct["msgp"][:, gl, 0:136],
                        start=(done == 0), stop=(done == tot - 1))
                    done += 1
            rec = pBs.tile([128, 8], fp32, tag="rec")
            nc.vector.reciprocal(out=rec[:], in_=ps_blk[:, 128:136])
            gt = pC.tile([128, 128], fp32, tag="gt")
            nc.vector.tensor_tensor(
                out=gt[:].rearrange("p (h f) -> p h f", f=F),
                in0=ps_blk[:, 0:128].rearrange("p (h f) -> p h f", f=F),
                in1=rec[:].to_broadcast([128, 8, F]), op=Alu.mult)
            if not host["triv_bgat"]:
                nc.vector.tensor_tensor(out=gt[:], in0=gt[:], in1=gl_s["bgat"][:],
                                        op=Alu.add)
            # ---- phase C for tile b ----
            xo = pC.tile([128, 128], fp32, tag="xo")
            nc.scalar.dma_start(out=xo[:], in_=xown_d[b * 128:(b + 1) * 128, :])
            t1 = pC.tile([128, 128], fp32, tag="t1")
            nc.vector.tensor_tensor(out=t1[:], in0=xo[:], in1=gt[:], op=Alu.add)

            def layer_norm(tin, g_key, b_key, triv, tagp):
                bst = pBs.tile([128, 6], fp32, tag=f"bst{tagp}")
                nc.vector.bn_stats(out=bst[:], in_=tin[:])
                mv = pBs.tile([128, 2], fp32, tag=f"mv{tagp}")
                nc.vector.bn_aggr(out=mv[:], in_=bst[:])
                # rstd = exp(-0.5 * ln(var + eps)); stays in the exp/ln set
                nc.scalar.activation(out=mv[:, 1:2], in_=mv[:, 1:2],
                                     func=Act.Ln, bias=eps_s[:])
                nc.scalar.activation(out=mv[:, 1:2], in_=mv[:, 1:2],
                                     func=Act.Exp, scale=-0.5)
                o = pC.tile([128, 128], fp32, tag=f"ln{tagp}")
                nc.vector.tensor_scalar(out=o[:], in0=tin[:],
                                        scalar1=mv[:, 0:1], op0=Alu.subtract,
                                        scalar2=mv[:, 1:2], op1=Alu.mult)
                if not triv:
                    nc.vector.tensor_tensor(out=o[:], in0=o[:], in1=gl_s[g_key][:],
                                            op=Alu.mult)
                    nc.vector.tensor_tensor(out=o[:], in0=o[:], in1=gl_s[b_key][:],
                                            op=Alu.add)
                return o

            u = layer_norm(t1, "g1", "b1", host["triv_gb1"], "1")
            u_bf = pC.tile([128, 128], bft, tag="ubf")
            nc.scalar.activation(out=u_bf[:], in_=u[:], func=Act.Copy)
            uT_ps = psC.tile([128, 128], bft, tag="uT")
            nc.tensor.transpose(uT_ps[:], in_=u_bf[:], identity=I128_s[:])
            uT = pC.tile([128, 128], bft, tag="uTs")
            nc.scalar.activation(out=uT[:], in_=uT_ps[:], func=Act.Copy)
            f1ps = psC.tile([128, 2, 128], fp32, tag="f1")
            for j in range(2):
                nc.tensor.matmul(f1ps[:, j, :], lhsT=W1_s[:, j * 128:(j + 1) * 128],
                                 rhs=uT[:], start=True, stop=True)
            r1 = pC.tile([128, 2, 128], bft, tag="r1")
            for j in range(2):
                nc.scalar.activation(out=r1[:, j, :], in_=f1ps[:, j, :],
                                     func=Act.Relu, bias=b1c_s[:, j:j + 1])
            zps = psC.tile([128, 128], fp32, tag="zp")
            for j in range(2):
                nc.tensor.matmul(zps[:], lhsT=r1[:, j, :], rhs=W2_s[:, j, :],
                                 start=(j == 0), stop=(j == 1))
            t2 = pC.tile([128, 128], fp32, tag="t2")
            nc.vector.tensor_tensor(out=t2[:], in0=u[:], in1=zps[:], op=Alu.add)
            if not host["triv_bff2"]:
                nc.vector.tensor_tensor(out=t2[:], in0=t2[:], in1=gl_s["bff2"][:],
                                        op=Alu.add)
            zt = layer_norm(t2, "g2", "b2", host["triv_gb2"], "2")
            nc.sync.dma_start(out=z_d[b * 128:(b + 1) * 128, :], in_=zt[:])

        for p in (psC, pC, psB, pBs, pB):
            p.release()
        cpool.release()

    nc.compile()
    return nc


def kernel(**inputs):
    from concourse.bass_utils import run_bass_kernel_spmd
    import os

    host = _build_host_data(inputs)
    nc = _build_program(host)

    in_maps = []
    for c in range(NCORES):
        m = {
            "xT": host["xT"][c],
            "x_own": host["x_own"][c],
            "Wp": host["Wp"], "I128": host["I128"], "onehotc": host["onehotc"],
            "W1": host["W1"], "W2": host["W2"], "b1col": host["b1col"],
        }
        if not host["triv_bgat"]:
            m["bgat_r"] = np.tile(host["bias_gat"].reshape(1, -1), (128, 1))
        if not host["triv_bff2"]:
            m["bff2_r"] = np.tile(host["b_ff2"].reshape(1, -1), (128, 1))
        if not host["triv_gb1"]:
            m["g1_r"] = np.tile(host["gamma1"].reshape(1, -1), (128, 1))
            m["b1_r"] = np.tile(host["beta1"].reshape(1, -1), (128, 1))
        if not host["triv_gb2"]:
            m["g2_r"] = np.tile(host["gamma2"].reshape(1, -1), (128, 1))
            m["b2_r"] = np.tile(host["beta2"].reshape(1, -1), (128, 1))
        for s, sname in ((0, "lo"), (1, "hi")):
            m[f"idx_{sname}"] = host["per_core"][c][s]["idx"]
        in_maps.append(m)

    trace = bool(int(os.environ.get("GAT_TRACE", "0")))
    res = run_bass_kernel_spmd(nc, in_maps, core_ids=list(range(NCORES)),
                               trace=trace)
    if trace and res.exec_time_ns:
        print(f"HW exec time: {res.exec_time_ns} ns")
    if bool(int(os.environ.get("GAT_TIME", "0"))):
        try:
            from concourse.timeline_sim import TimelineSim
            ts = TimelineSim(nc)
            dur = ts.simulate()
            print(f"HW exec time: {dur:.0f} ns (cost-model timeline estimate)")
        except Exception as e:
            print("timeline sim failed:", e)

    out = np.zeros((N, D), np.float32)
    for c in range(NCORES):
        lo_n = OWN * c
        hi_n = min(OWN * (c + 1), N)
        out[lo_n:hi_n] = res.results[c]["z"][: hi_n - lo_n]
    return out


# revision 17
# speedup vs baseline: 1.3248x; 1.0326x over previous
"""Trainium2 Bass kernel for a GAT block (GATConv + LN + FFN + LN).

Self-contained: builds per-core shards on the host, compiles one SPMD Bass
program, runs it on 8 NeuronCores via run_bass_kernel_spmd, and reassembles
the full [50000, 128] output.

Per-core scheme (core c of 8, nodes permuted own|zeropad|rest):
  Phase A: for all 50304 (padded) rows compute [h | a_src | a_dst] =
           x @ [W | W@Asrc | W@Adst] on PE; store 512-byte node rows
           [h(128) | a_src(8) | a_dst(8) | onehot_fp8(64 bf16 slots) | pad]
           to core-local DRAM. The onehot block is a constant identity
           pattern (row r holds onehot(r % 128) in fp8) that later serves
           as ready-made scatter-matmul weights.
  Phase B: edges with dst owned by the core (incl self-loops), grouped by
           128-node dst block, split lo/hi on the 32K int16 gather-index
           limit, padded per (block, stream) to 128-edge granules with a
           shared max-over-cores profile so all cores run one program.
           Per 4096-edge chunk: gather#1 512B src rows (h + a_src);
           gather#2 256B dst meta half-rows (a_dst + fp8 onehot = S);
           eL = a_src + a_dst; p = exp(leaky_relu(eL)); pexp = head-expand
           of p on the Act engine; msg = h * pexp (DVE 2x); per granule:
           psum[block] += S_fp8^T-matmul [msg | p].
  Phase C: g = agg/denom; u = LN(x + g); ff = relu(u@W1 + b1)@W2 + b2;
           z = LN(u + ff). rstd via exp(-0.5*ln(var+eps)) keeps every
           activation in one table set (no LoadActFuncSet churn).
"""
import numpy as np
import ml_dtypes

N = 50000
NCORES = 8
OWN = 6272             # nodes per core (49 tiles of 128)
ZPAD = 128             # zero rows after own block (pad-edge target)
NPG = OWN * NCORES     # globally padded node count (50176)
NP2 = NPG + ZPAD       # per-core row count (50304)
BLK = 128
NBLK = OWN // BLK      # 49
GR = 128               # edges per granule
CHUNK = 4096
GPC = CHUNK // GR      # 32
LO_LIM = 1 << 15
H, F, D = 8, 16, 128
ROW = 256              # bf16 cols per node row (512 bytes)
C_AS = 128             # a_src col
C_AD = 136             # a_dst col
C_OH = 144             # onehot (fp8-as-bf16) col, 64 cols
LN_EPS = 1e-5

bf16 = ml_dtypes.bfloat16
f8 = ml_dtypes.float8_e4m3fn


def _wrapc(idx):
    """Per-chunk 16-wrap: [CHUNK] int -> [128, CHUNK//16] int16."""
    w = idx.reshape(CHUNK // 16, 16).T.astype(np.int16)
    return np.tile(w, (8, 1))


def _bfr(x):
    return np.ascontiguousarray(x, dtype=np.float32).astype(bf16)


def _build_host_data(inputs):
    x = np.asarray(inputs["x"], np.float32)
    W = np.asarray(inputs["W_gat"], np.float32)
    att_src = np.asarray(inputs["att_src"], np.float32)
    att_dst = np.asarray(inputs["att_dst"], np.float32)
    ei = np.asarray(inputs["edge_index"])

    src = ei[0].astype(np.int64)
    dst = ei[1].astype(np.int64)
    loops = np.arange(N, dtype=np.int64)
    src = np.concatenate([src, loops])
    dst = np.concatenate([dst, loops])

    # per-core row index of global padded node g:
    #   own -> [0, OWN); zeros -> [OWN, OWN+ZPAD); rest keeps order after
    def inv_row(c, g):
        own = (g >= OWN * c) & (g < OWN * (c + 1))
        r = np.where(own, g - OWN * c,
                     OWN + ZPAD + np.where(g < OWN * c, g, g - OWN))
        return r

    # per (core, block, stream) counts on row-mapped gather indices
    counts = np.zeros((NCORES, NBLK, 2), dtype=np.int64)
    core_edges = []
    for c in range(NCORES):
        m = (dst >= OWN * c) & (dst < min(OWN * (c + 1), N))
        s_g = inv_row(c, src[m])
        d_l = dst[m] - OWN * c
        blk = d_l // BLK
        lo = s_g < LO_LIM
        core_edges.append((s_g, d_l, blk, lo))
        for b in range(NBLK):
            mb = blk == b
            counts[c, b, 0] = np.sum(mb & lo)
            counts[c, b, 1] = np.sum(mb & ~lo)

    g_prof = np.ceil(counts.max(axis=0) / GR).astype(np.int64)   # [NBLK, 2]
    L = [int(g_prof[:, s].sum()) * GR for s in range(2)]
    for s in range(2):
        pad = (-L[s]) % CHUNK
        g_prof[NBLK - 1, s] += pad // GR
        L[s] += pad
    L_LO, L_HI = L

    per_core = []
    for c in range(NCORES):
        s_g, d_l, blk, lo = core_edges[c]
        streams = []
        for sidx in range(2):
            mm = lo if sidx == 0 else ~lo
            Ls = L[sidx]
            gidx = np.zeros(Ls, dtype=np.int64)
            aidx = np.full(Ls, OWN, dtype=np.int64)   # pads -> zero row
            pos = 0
            for b in range(NBLK):
                mb = (blk == b) & mm
                k = int(np.sum(mb))
                cap = int(g_prof[b, sidx]) * GR
                gidx[pos:pos + k] = s_g[mb] - (0 if sidx == 0 else LO_LIM)
                aidx[pos:pos + k] = d_l[mb]
                pos += cap
            # pack [gidx | aidx] wrapped per chunk: [128, nch*512] i16
            nch = Ls // CHUNK
            pk = np.zeros((128, nch * 512), dtype=np.int16)
            for k in range(nch):
                pk[:, k * 512:k * 512 + 256] = _wrapc(gidx[k * CHUNK:(k + 1) * CHUNK])
                pk[:, k * 512 + 256:(k + 1) * 512] = _wrapc(aidx[k * CHUNK:(k + 1) * CHUNK])
            streams.append({"idx": np.ascontiguousarray(pk)})
        per_core.append(streams)

    # weights: Wp = [W | W@Asrc | W@Adst]  -> [128, 144]
    Asrc = np.zeros((D, H), np.float32)
    Adst = np.zeros((D, H), np.float32)
    for h in range(H):
        Asrc[h * F:(h + 1) * F, h] = att_src[h]
        Adst[h * F:(h + 1) * F, h] = att_dst[h]
    Wp = _bfr(np.concatenate([W, W @ Asrc, W @ Adst], axis=1))   # [128, 144]
    I128 = _bfr(np.eye(128, dtype=np.float32))
    # onehot identity as fp8 bytes viewed as bf16: [128, 64]
    oh = np.zeros((128, 128), dtype=f8)
    for i in range(128):
        oh[i, i] = 1.0
    onehotc = np.ascontiguousarray(oh).view(np.uint16).view(bf16)  # [128, 64]

    xp = np.zeros((NP2, D), np.float32)
    xp[:N] = x                         # global padded layout first
    xT_per_core = []
    x_own_per_core = []
    for c in range(NCORES):
        rows = np.zeros((NP2, D), np.float32)
        rows[0:OWN] = xp[OWN * c: OWN * (c + 1)]
        rest = np.concatenate([xp[: OWN * c], xp[OWN * (c + 1): NPG]])
        rows[OWN + ZPAD:] = rest
        xT_per_core.append(np.ascontiguousarray(rows.T.astype(bf16)))
        x_own_per_core.append(np.ascontiguousarray(xp[OWN * c: OWN * (c + 1)]))

    host = {
        "g_prof": g_prof, "L_LO": L_LO, "L_HI": L_HI,
        "per_core": per_core, "xT": xT_per_core, "x_own": x_own_per_core,
        "Wp": Wp, "I128": I128, "onehotc": onehotc,
        "W1": _bfr(np.asarray(inputs["w_ff1"], np.float32)),
        "W2": _bfr(np.asarray(inputs["w_ff2"], np.float32)),
        "b1col": np.ascontiguousarray(
            np.asarray(inputs["b_ff1"], np.float32).reshape(2, 128).T),
    }
    host["bias_gat"] = np.asarray(inputs["bias_gat"], np.float32)
    host["b_ff2"] = np.asarray(inputs["b_ff2"], np.float32)
    for nm in ("gamma1", "beta1", "gamma2", "beta2"):
        host[nm] = np.asarray(inputs[nm], np.float32)
    host["triv_gb1"] = bool(np.all(host["gamma1"] == 1) and np.all(host["beta1"] == 0))
    host["triv_gb2"] = bool(np.all(host["gamma2"] == 1) and np.all(host["beta2"] == 0))
    host["triv_bgat"] = bool(np.all(host["bias_gat"] == 0))
    host["triv_bff2"] = bool(np.all(host["b_ff2"] == 0))
    return host


def _build_program(host):
    import concourse.bacc as bacc
    import concourse.mybir as mybir
    import concourse.tile as tile

    fp32 = mybir.dt.float32
    bft = mybir.dt.bfloat16
    i16 = mybir.dt.int16
    fp8 = mybir.dt.float8e4
    Alu = mybir.AluOpType
    Act = mybir.ActivationFunctionType

    g_prof = host["g_prof"]
    L_LO, L_HI = host["L_LO"], host["L_HI"]

    nc = bacc.Bacc("TRN2")

    # Pre-place one activation-table load that covers every func we use, so
    # the compile-time fixpoint pass never needs to thrash between sets.
    from concourse.hw_specs import get_activation_tables
    _tabs = list(get_activation_tables(nc.m.arch).items())
    _need = {Act.Exp, Act.Ln, Act.Copy, Act.Relu}
    _set_id = next(i for i, (_n, fns) in enumerate(_tabs) if _need <= fns)

    xT_d = nc.dram_tensor("xT", [128, NP2], bft, kind="ExternalInput")
    xown_d = nc.dram_tensor("x_own", [OWN, D], fp32, kind="ExternalInput")
    Wp_d = nc.dram_tensor("Wp", [128, 144], bft, kind="ExternalInput")
    I128_d = nc.dram_tensor("I128", [128, 128], bft, kind="ExternalInput")
    oh_d = nc.dram_tensor("onehotc", [128, 64], bft, kind="ExternalInput")
    W1_d = nc.dram_tensor("W1", [128, 256], bft, kind="ExternalInput")
    W2_d = nc.dram_tensor("W2", [256, 128], bft, kind="ExternalInput")
    b1c_d = nc.dram_tensor("b1col", [128, 2], fp32, kind="ExternalInput")
    gl_d = {}
    if not host["triv_bgat"]:
        gl_d["bgat"] = nc.dram_tensor("bgat_r", [128, 128], fp32, kind="ExternalInput")
    if not host["triv_bff2"]:
        gl_d["bff2"] = nc.dram_tensor("bff2_r", [128, 128], fp32, kind="ExternalInput")
    if not host["triv_gb1"]:
        gl_d["g1"] = nc.dram_tensor("g1_r", [128, 128], fp32, kind="ExternalInput")
        gl_d["b1"] = nc.dram_tensor("b1_r", [128, 128], fp32, kind="ExternalInput")
    if not host["triv_gb2"]:
        gl_d["g2"] = nc.dram_tensor("g2_r", [128, 128], fp32, kind="ExternalInput")
        gl_d["b2"] = nc.dram_tensor("b2_r", [128, 128], fp32, kind="ExternalInput")

    st_d = []
    for sname, Ls in (("lo", L_LO), ("hi", L_HI)):
        st_d.append({
            "idx": nc.dram_tensor(f"idx_{sname}", [128, (Ls // CHUNK) * 512], i16,
                                  kind="ExternalInput"),
            "L": Ls,
        })

    h_d = nc.dram_tensor("h_scratch", [NP2, ROW], bft, kind="Internal")
    z_d = nc.dram_tensor("z", [OWN, D], fp32, kind="ExternalOutput")

    NT2 = NP2 // 128                  # 393 node tiles
    PADT = OWN // 128                 # tile 49 == the zero-pad block
    GT = 3                            # node tiles per psum bank
    SW = 12                           # node tiles per stage flush / x DMA

    with tile.TileContext(nc) as tc:
        nc.scalar.add_instruction(mybir.InstLoadActFuncSet(
            name=nc.get_next_instruction_name(), ins=[], outs=[],
            act_func_set_id=_set_id))
        # ================= consts =================
        cpool = tc.alloc_tile_pool(name="consts", bufs=1)
        Wp_s = cpool.tile([128, 144], bft)
        nc.sync.dma_start(out=Wp_s[:], in_=Wp_d[:])
        I128_s = cpool.tile([128, 128], bft)
        nc.sync.dma_start(out=I128_s[:], in_=I128_d[:])
        W1_s = cpool.tile([128, 256], bft)
        nc.sync.dma_start(out=W1_s[:], in_=W1_d[:])
        W2_s = cpool.tile([256 // 2, 2, 128], bft)
        nc.sync.dma_start(out=W2_s[:],
                          in_=W2_d[:].rearrange("(k h) f -> h k f", k=2))
        b1c_s = cpool.tile([128, 2], fp32)
        nc.sync.dma_start(out=b1c_s[:], in_=b1c_d[:])
        gl_s = {}
        for k, dref in gl_d.items():
            gl_s[k] = cpool.tile([128, 128], fp32, tag=f"gl_{k}")
            nc.sync.dma_start(out=gl_s[k][:], in_=dref[:])
        eps_s = cpool.tile([128, 1], fp32)
        nc.vector.memset(eps_s[:], LN_EPS)
        zt64 = cpool.tile([128, 64], bft)
        nc.vector.memset(zt64[:], 0.0)
        # two fixed stage buffers with persistent onehot-const region
        stgpool = tc.alloc_tile_pool(name="stg", bufs=1)
        stg = []
        for i in range(2):
            s = stgpool.tile([128, SW, ROW], bft, tag=f"stage{i}")
            nc.vector.memset(s[:], 0.0)
            nc.sync.dma_start(
                out=s[:, :, C_OH:C_OH + 64],
                in_=oh_d[:].rearrange("p (o f) -> p o f", o=1).to_broadcast(
                    [128, SW, 64]))
            stg.append(s)

        # ================= phase A =================
        with tc.tile_pool(name="pA", bufs=4) as pA, \
             tc.tile_pool(name="psA", bufs=4, space="PSUM") as psA:
            xt = None
            for tg in range((NT2 + GT - 1) // GT):
                t0 = tg * GT
                ntl = min(GT, NT2 - t0)
                if t0 % SW == 0:
                    nxb = min(SW, NT2 - t0)
                    xt = pA.tile([128, SW * 128], bft, tag="xt")
                    nc.sync.dma_start(out=xt[:, :nxb * 128],
                                      in_=xT_d[:, t0 * 128:(t0 + nxb) * 128])
                ps = psA.tile([128, GT, 144], fp32, tag="psA")
                for j in range(ntl):
                    jo = (t0 % SW) + j
                    nc.tensor.matmul(ps[:, j, :],
                                     lhsT=xt[:, jo * 128:(jo + 1) * 128],
                                     rhs=Wp_s[:], start=True, stop=True)
                sb = stg[(t0 // SW) % 2]
                j0 = t0 % SW
                if tg % 2 == 0:
                    nc.scalar.activation(out=sb[:, j0:j0 + ntl, 0:144],
                                         in_=ps[:, :ntl, :], func=Act.Copy)
                else:
                    nc.vector.tensor_copy(out=sb[:, j0:j0 + ntl, 0:144],
                                          in_=ps[:, :ntl, :])
                if j0 + ntl == SW or t0 + ntl == NT2:
                    nf = j0 + ntl
                    r0 = (t0 + ntl - nf) * 128
                    nc.sync.dma_start(
                        out=h_d[r0:r0 + nf * 128, :].rearrange(
                            "(j n) d -> n j d", j=nf),
                        in_=sb[:, :nf, :])
            # zero the onehot region of the zero-pad block rows
            nc.sync.dma_start(out=h_d[OWN:OWN + ZPAD, C_OH:C_OH + 64],
                              in_=zt64[:])

        tc.strict_bb_all_engine_barrier()
        stgpool.release()

        # ================= phases B + C =================
        h_lo = h_d[0:LO_LIM, :]
        h_hi = h_d[LO_LIM:NP2, :]
        meta_tab = h_d[:, 128:256]     # [NP2, 128] at 512B pitch
        starts = np.zeros((NBLK, 2), dtype=np.int64)
        for s in range(2):
            starts[1:, s] = np.cumsum(g_prof[:-1, s])

        pB = tc.alloc_tile_pool(name="pB", bufs=5)
        pBs = tc.alloc_tile_pool(name="pBsmall", bufs=6)
        psB = tc.alloc_tile_pool(name="psB", bufs=4, space="PSUM")
        pC = tc.alloc_tile_pool(name="pC", bufs=2)
        psC = tc.alloc_tile_pool(name="psC", bufs=1, space="PSUM")

        chunk_tiles = [{}, {}]

        def emit_chunk(s, k, prefetch=True):
            if k in chunk_tiles[s]:
                return chunk_tiles[s][k]
            sd = st_d[s]
            idx = pBs.tile([128, 512], i16, tag="idx")
            nc.sync.dma_start(out=idx[:],
                              in_=sd["idx"][:, k * 512:(k + 1) * 512])
            hrow = pB.tile([128, GPC, ROW], bft, tag="h")
            nc.gpsimd.dma_gather(hrow[:], h_lo if s == 0 else h_hi,
                                 idx[:, 0:256], CHUNK, CHUNK, ROW,
                                 single_packet=False)
            meta = pB.tile([128, GPC, 128], bft, tag="m")
            nc.gpsimd.dma_gather(meta[:], meta_tab, idx[:, 256:512],
                                 CHUNK, CHUNK, 128, elem_step=ROW,
                                 single_packet=False)
            # eL = a_src[src] + a_dst[dst]
            eL = pBs.tile([128, GPC, 8], bft, tag="eL")
            nc.vector.tensor_tensor(out=eL[:], in0=hrow[:, :, C_AS:C_AS + 8],
                                    in1=meta[:, :, 8:16], op=Alu.add)
            eL2 = pBs.tile([128, GPC, 8], bft, tag="eL2")
            nc.vector.scalar_tensor_tensor(out=eL2[:], in0=eL[:], scalar=0.2,
                                           in1=eL[:], op0=Alu.mult, op1=Alu.max)
            # p over the spent a_src cols; msg in place over h
            nc.scalar.activation(out=hrow[:, :, 128:136], in_=eL2[:],
                                 func=Act.Exp)
            pexp = pB.tile([128, GPC, 128], bft, tag="px")
            nc.scalar.activation(
                out=pexp[:].rearrange("p g (h f) -> p g h f", f=F),
                in_=eL2[:].to_broadcast([128, GPC, 8, F]), func=Act.Exp)
            nc.vector.tensor_tensor(out=hrow[:, :, 0:128],
                                    in0=hrow[:, :, 0:128], in1=pexp[:],
                                    op=Alu.mult)
            res = {"S": meta, "msgp": hrow}
            chunk_tiles[s][k] = res
            # prefetch one chunk ahead (non-recursive) so its gathers are in
            # the instruction stream before this chunk's consumers
            if prefetch and (k + 1) * CHUNK < sd["L"]:
                emit_chunk(s, k + 1, prefetch=False)
            return res

        for b in range(NBLK):
            ps_blk = psB.tile([128, 136], fp32, tag="blk")
            tot = int(g_prof[b, 0] + g_prof[b, 1])
            done = 0
            for s in range(2):
                for gi in range(int(g_prof[b, s])):
                    gg = int(starts[b, s]) + gi
                    ct = emit_chunk(s, gg // GPC)
                    gl = gg % GPC
                    nc.tensor.matmul(
                        ps_blk[:],
                        lhsT=ct["S"][:, gl, 16:80].bitcast(fp8),
                        rhs=ct["msgp"][:, gl, 0:136],
                        start=(done == 0), stop=(done == tot - 1))
                    done += 1
            rec = pBs.tile([128, 8], fp32, tag="rec")
            nc.vector.reciprocal(out=rec[:], in_=ps_blk[:, 128:136])
            gt = pC.tile([128, 128], fp32, tag="gt")
            nc.vector.tensor_tensor(
                out=gt[:].rearrange("p (h f) -> p h f", f=F),
                in0=ps_blk[:, 0:128].rearrange("p (h f) -> p h f", f=F),
                in1=rec[:].to_broadcast([128, 8, F]), op=Alu.mult)
            if not host["triv_bgat"]:
                nc.vector.tensor_tensor(out=gt[:], in0=gt[:], in1=gl_s["bgat"][:],
                                        op=Alu.add)
            # ---- phase C for tile b ----
            xo = pC.tile([128, 128], fp32, tag="xo")
            nc.sync.dma_start(out=xo[:], in_=xown_d[b * 128:(b + 1) * 128, :])
            t1 = pC.tile([128, 128], fp32, tag="t1")
            nc.vector.tensor_tensor(out=t1[:], in0=xo[:], in1=gt[:], op=Alu.add)

            def layer_norm(tin, g_key, b_key, triv, tagp):
                bst = pBs.tile([128, 6], fp32, tag=f"bst{tagp}")
                nc.vector.bn_stats(out=bst[:], in_=tin[:])
                mv = pBs.tile([128, 2], fp32, tag=f"mv{tagp}")
                nc.vector.bn_aggr(out=mv[:], in_=bst[:])
                # rstd = exp(-0.5 * ln(var + eps)); stays in the exp/ln set
                nc.scalar.activation(out=mv[:, 1:2], in_=mv[:, 1:2],
                                     func=Act.Ln, bias=eps_s[:])
                nc.scalar.activation(out=mv[:, 1:2], in_=mv[:, 1:2],
                                     func=Act.Exp, scale=-0.5)
                o = pC.tile([128, 128], fp32, tag=f"ln{tagp}")
                nc.vector.tensor_scalar(out=o[:], in0=tin[:],
                                        scalar1=mv[:, 0:1], op0=Alu.subtract,
                                        scalar2=mv[:, 1:2], op1=Alu.mult)
                if not triv:
                    nc.vector.tensor_tensor(out=o[:], in0=o[:], in1=gl_s[g_key][:],
                                            op=Alu.mult)
                    nc.vector.tensor_tensor(out=o[:], in0=o[:], in1=gl_s[b_key][:],
                                            op=Alu.add)
                return o

            u = layer_norm(t1, "g1", "b1", host["triv_gb1"], "1")
            u_bf = pC.tile([128, 128], bft, tag="ubf")
            nc.scalar.activation(out=u_bf[:], in_=u[:], func=Act.Copy)
            uT_ps = psC.tile([128, 128], bft, tag="uT")
            nc.tensor.transpose(uT_ps[:], in_=u_bf[:], identity=I128_s[:])
            uT = pC.tile([128, 128], bft, tag="uTs")
            nc.scalar.activation(out=uT[:], in_=uT_ps[:], func=Act.Copy)
            f1ps = psC.tile([128, 2, 128], fp32, tag="f1")
            for j in range(2):
                nc.tensor.matmul(f1ps[:, j, :], lhsT=W1_s[:, j * 128:(j + 1) * 128],
                                 rhs=uT[:], start=True, stop=True)
            r1 = pC.tile([128, 2, 128], bft, tag="r1")
            for j in range(2):
                nc.scalar.activation(out=r1[:, j, :], in_=f1ps[:, j, :],
                                     func=Act.Relu, bias=b1c_s[:, j:j + 1])
            zps = psC.tile([128, 128], fp32, tag="zp")
            for j in range(2):
                nc.tensor.matmul(zps[:], lhsT=r1[:, j, :], rhs=W2_s[:, j, :],
                                 start=(j == 0), stop=(j == 1))
            t2 = pC.tile([128, 128], fp32, tag="t2")
            nc.vector.tensor_tensor(out=t2[:], in0=u[:], in1=zps[:], op=Alu.add)
            if not host["triv_bff2"]:
                nc.vector.tensor_tensor(out=t2[:], in0=t2[:], in1=gl_s["bff2"][:],
                                        op=Alu.add)
            zt = layer_norm(t2, "g2", "b2", host["triv_gb2"], "2")
            nc.sync.dma_start(out=z_d[b * 128:(b + 1) * 128, :], in_=zt[:])

        for p in (psC, pC, psB, pBs, pB):
            p.release()
        cpool.release()

    nc.compile()
    return nc


def kernel(**inputs):
    from concourse.bass_utils import run_bass_kernel_spmd
    import os

    host = _build_host_data(inputs)
    nc = _build_program(host)

    in_maps = []
    for c in range(NCORES):
        m = {
            "xT": host["xT"][c],
            "x_own": host["x_own"][c],
            "Wp": host["Wp"], "I128": host["I128"], "onehotc": host["onehotc"],
            "W1": host["W1"], "W2": host["W2"], "b1col": host["b1col"],
        }
        if not host["triv_bgat"]:
            m["bgat_r"] = np.tile(host["bias_gat"].reshape(1, -1), (128, 1))
        if not host["triv_bff2"]:
            m["bff2_r"] = np.tile(host["b_ff2"].reshape(1, -1), (128, 1))
        if not host["triv_gb1"]:
            m["g1_r"] = np.tile(host["gamma1"].reshape(1, -1), (128, 1))
            m["b1_r"] = np.tile(host["beta1"].reshape(1, -1), (128, 1))
        if not host["triv_gb2"]:
            m["g2_r"] = np.tile(host["gamma2"].reshape(1, -1), (128, 1))
            m["b2_r"] = np.tile(host["beta2"].reshape(1, -1), (128, 1))
        for s, sname in ((0, "lo"), (1, "hi")):
            m[f"idx_{sname}"] = host["per_core"][c][s]["idx"]
        in_maps.append(m)

    trace = bool(int(os.environ.get("GAT_TRACE", "0")))
    res = run_bass_kernel_spmd(nc, in_maps, core_ids=list(range(NCORES)),
                               trace=trace)
    if trace and res.exec_time_ns:
        print(f"HW exec time: {res.exec_time_ns} ns")
    if bool(int(os.environ.get("GAT_TIME", "0"))):
        try:
            from concourse.timeline_sim import TimelineSim
            ts = TimelineSim(nc)
            dur = ts.simulate()
            print(f"HW exec time: {dur:.0f} ns (cost-model timeline estimate)")
        except Exception as e:
            print("timeline sim failed:", e)

    out = np.zeros((N, D), np.float32)
    for c in range(NCORES):
        lo_n = OWN * c
        hi_n = min(OWN * (c + 1), N)
        out[lo_n:hi_n] = res.results[c]["z"][: hi_n - lo_n]
    return out


# revision 30
# speedup vs baseline: 1.5972x; 1.2057x over previous
"""Trainium2 Bass kernel for a GAT block (GATConv + LN + FFN + LN).

Self-contained: builds per-core shards on the host, compiles one SPMD Bass
program, runs it on 8 NeuronCores via run_bass_kernel_spmd, and reassembles
the full [50000, 128] output.

Per-core scheme (core c of 8, nodes permuted own|zeropad|rest):
  Phase A: for all 50304 (padded) rows compute [h | a_src | a_dst] =
           x @ [W | W@Asrc | W@Adst] on PE; store 512-byte node rows
           [h(128) | a_src(8) | a_dst(8) | onehot_fp8(64 bf16 slots) | pad]
           to core-local DRAM. The onehot block is a constant identity
           pattern (row r holds onehot(r % 128) in fp8) that later serves
           as ready-made scatter-matmul weights.
  Phase B: edges with dst owned by the core (incl self-loops), grouped by
           128-node dst block, split lo/hi on the 32K int16 gather-index
           limit, padded per (block, stream) to 128-edge granules with a
           shared max-over-cores profile so all cores run one program.
           Per 4096-edge chunk: gather#1 512B src rows (h + a_src);
           gather#2 256B dst meta half-rows (a_dst + fp8 onehot = S);
           eL = a_src + a_dst; p = exp(leaky_relu(eL)); pexp = head-expand
           of p on the Act engine; msg = h * pexp (DVE 2x); per granule:
           psum[block] += S_fp8^T-matmul [msg | p].
  Phase C: g = agg/denom; u = LN(x + g); ff = relu(u@W1 + b1)@W2 + b2;
           z = LN(u + ff). rstd via exp(-0.5*ln(var+eps)) keeps every
           activation in one table set (no LoadActFuncSet churn).
"""
import numpy as np
import ml_dtypes

N = 50000
NCORES = 8
OWN = 6272             # nodes per core (49 tiles of 128)
ZPAD = 128             # zero rows after own block (pad-edge target)
NPG = OWN * NCORES     # globally padded node count (50176)
NP2 = NPG + ZPAD       # per-core row count (50304)
BLK = 128
NBLK = OWN // BLK      # 49
GR = 128               # edges per granule
CHUNK = 4096
GPC = CHUNK // GR      # 32
LO_LIM = 1 << 15
H, F, D = 8, 16, 128
ROW = 256              # bf16 cols per node row (512 bytes)
C_AS = 128             # a_src col
C_AD = 136             # a_dst col
C_OH = 144             # onehot (fp8-as-bf16) col, 64 cols
LN_EPS = 1e-5

bf16 = ml_dtypes.bfloat16
f8 = ml_dtypes.float8_e4m3fn


def _wrapc(idx):
    """Per-chunk 16-wrap: [CHUNK] int -> [128, CHUNK//16] int16."""
    w = idx.reshape(CHUNK // 16, 16).T.astype(np.int16)
    return np.tile(w, (8, 1))


def _bfr(x):
    return np.ascontiguousarray(x, dtype=np.float32).astype(bf16)


def _build_host_data(inputs):
    x = np.asarray(inputs["x"], np.float32)
    W = np.asarray(inputs["W_gat"], np.float32)
    att_src = np.asarray(inputs["att_src"], np.float32)
    att_dst = np.asarray(inputs["att_dst"], np.float32)
    ei = np.asarray(inputs["edge_index"])

    src = ei[0].astype(np.int64)
    dst = ei[1].astype(np.int64)
    loops = np.arange(N, dtype=np.int64)
    src = np.concatenate([src, loops])
    dst = np.concatenate([dst, loops])

    # per-core row index of global padded node g:
    #   own -> [0, OWN); zeros -> [OWN, OWN+ZPAD); rest keeps order after
    def inv_row(c, g):
        own = (g >= OWN * c) & (g < OWN * (c + 1))
        r = np.where(own, g - OWN * c,
                     OWN + ZPAD + np.where(g < OWN * c, g, g - OWN))
        return r

    # per (core, block, stream) counts on row-mapped gather indices
    counts = np.zeros((NCORES, NBLK, 2), dtype=np.int64)
    core_edges = []
    for c in range(NCORES):
        m = (dst >= OWN * c) & (dst < min(OWN * (c + 1), N))
        s_g = inv_row(c, src[m])
        d_l = dst[m] - OWN * c
        blk = d_l // BLK
        lo = s_g < LO_LIM
        core_edges.append((s_g, d_l, blk, lo))
        for b in range(NBLK):
            mb = blk == b
            counts[c, b, 0] = np.sum(mb & lo)
            counts[c, b, 1] = np.sum(mb & ~lo)

    g_prof = np.ceil(counts.max(axis=0) / GR).astype(np.int64)   # [NBLK, 2]
    L = [int(g_prof[:, s].sum()) * GR for s in range(2)]
    for s in range(2):
        pad = (-L[s]) % CHUNK
        g_prof[NBLK - 1, s] += pad // GR
        L[s] += pad
    L_LO, L_HI = L

    per_core = []
    for c in range(NCORES):
        s_g, d_l, blk, lo = core_edges[c]
        streams = []
        for sidx in range(2):
            mm = lo if sidx == 0 else ~lo
            Ls = L[sidx]
            gidx = np.zeros(Ls, dtype=np.int64)
            aidx = np.full(Ls, OWN, dtype=np.int64)   # pads -> zero row
            dl = np.full(Ls, -1, dtype=np.int64)      # pads -> no S entry
            pos = 0
            for b in range(NBLK):
                mb = (blk == b) & mm
                k = int(np.sum(mb))
                cap = int(g_prof[b, sidx]) * GR
                gidx[pos:pos + k] = s_g[mb] - (0 if sidx == 0 else LO_LIM)
                aidx[pos:pos + k] = d_l[mb]
                dl[pos:pos + k] = d_l[mb] % BLK
                pos += cap
            # pack [gidx | aidx] wrapped per chunk: [128, nch*512] i16
            nch = Ls // CHUNK
            pk = np.zeros((128, nch * 512), dtype=np.int16)
            for k in range(nch):
                pk[:, k * 512:k * 512 + 256] = _wrapc(gidx[k * CHUNK:(k + 1) * CHUNK])
                pk[:, k * 512 + 256:(k + 1) * 512] = _wrapc(aidx[k * CHUNK:(k + 1) * CHUNK])
            # S stream: per slot a 128-byte fp8 onehot(dl) row, laid out
            # [128 partitions(edge%128), nch*GPC*128 bytes] -> i16 view
            SB = np.zeros((Ls, 128), dtype=np.uint8)
            real = dl >= 0
            SB[np.nonzero(real)[0], dl[real]] = 0x38          # fp8 e4m3 1.0
            SB = SB.reshape(nch, GPC, 128, 128).transpose(2, 0, 1, 3)
            SB = np.ascontiguousarray(SB.reshape(128, nch * GPC * 128))
            streams.append({"idx": np.ascontiguousarray(pk),
                            "S": SB.view(np.int16)})
        per_core.append(streams)

    # weights: Wp = [W | W@Asrc | W@Adst]  -> [128, 144]
    Asrc = np.zeros((D, H), np.float32)
    Adst = np.zeros((D, H), np.float32)
    for h in range(H):
        Asrc[h * F:(h + 1) * F, h] = att_src[h]
        Adst[h * F:(h + 1) * F, h] = att_dst[h]
    Wp = _bfr(np.concatenate([W, W @ Asrc, W @ Adst], axis=1))   # [128, 144]
    I128 = _bfr(np.eye(128, dtype=np.float32))

    xp = np.zeros((NP2, D), np.float32)
    xp[:N] = x                         # global padded layout first
    xT_per_core = []
    x_own_per_core = []
    for c in range(NCORES):
        rows = np.zeros((NP2, D), np.float32)
        rows[0:OWN] = xp[OWN * c: OWN * (c + 1)]
        rest = np.concatenate([xp[: OWN * c], xp[OWN * (c + 1): NPG]])
        rows[OWN + ZPAD:] = rest
        xT_per_core.append(np.ascontiguousarray(rows.T.astype(bf16)))
        x_own_per_core.append(np.ascontiguousarray(xp[OWN * c: OWN * (c + 1)]))

    host = {
        "g_prof": g_prof, "L_LO": L_LO, "L_HI": L_HI,
        "per_core": per_core, "xT": xT_per_core, "x_own": x_own_per_core,
        "Wp": Wp, "I128": I128,
        "W1": _bfr(np.asarray(inputs["w_ff1"], np.float32)),
        "W2": _bfr(np.asarray(inputs["w_ff2"], np.float32)),
        "b1col": np.ascontiguousarray(
            np.asarray(inputs["b_ff1"], np.float32).reshape(2, 128).T),
    }
    host["bias_gat"] = np.asarray(inputs["bias_gat"], np.float32)
    host["b_ff2"] = np.asarray(inputs["b_ff2"], np.float32)
    for nm in ("gamma1", "beta1", "gamma2", "beta2"):
        host[nm] = np.asarray(inputs[nm], np.float32)
    host["triv_gb1"] = bool(np.all(host["gamma1"] == 1) and np.all(host["beta1"] == 0))
    host["triv_gb2"] = bool(np.all(host["gamma2"] == 1) and np.all(host["beta2"] == 0))
    host["triv_bgat"] = bool(np.all(host["bias_gat"] == 0))
    host["triv_bff2"] = bool(np.all(host["b_ff2"] == 0))
    return host


def _dma_gather_raw(eng, out_ap, in_ap, idxs_ap, num_idxs, elem_size,
                    elem_step):
    """dma_gather with elem_size below the 256-byte API floor (the floor is a
    transpose-mode restriction; plain row gathers take any size whose row
    pitch is a 256B multiple)."""
    import concourse.mybir as mybir
    from concourse import ap_utils

    assert idxs_ap.dtype == mybir.dt.int16
    assert in_ap.dtype == out_ap.dtype
    assert ap_utils.ap_is_contiguous(out_ap.ap[1:])
    assert ap_utils.ap_is_contiguous(idxs_ap.ap[1:])
    assert in_ap.ap[-1][1] == out_ap.ap[-1][1] == elem_size
    assert in_ap.ap[0][0] == elem_step
    stride_bytes = elem_step * mybir.dt.size(in_ap.dtype)
    assert stride_bytes % 256 == 0
    _in_ap = eng.lower_ap_dma(in_ap, for_custom_bir_dma=True)
    _idxs_ap = eng.lower_ap(idxs_ap)
    _out_ap = eng.lower_ap(out_ap)
    return eng.add_instruction(mybir.InstDMAGatherAnt(
        name=eng.bass.get_next_instruction_name(),
        ins=[*_in_ap, _idxs_ap, eng.lower_val_access(eng.to_reg(num_idxs))],
        outs=[_out_ap],
        transpose=False, num_idxs=num_idxs, elem_size=elem_size,
        stride_bytes_256=stride_bytes // 256, gen_mode=0,
        single_packet=False, queue_num=0,
        sbuf_tokens_per_rank=0, sbuf_free_dim_per_rank=0,
        sbuf_free_dim_pad_per_rank=0, sbuf_byte_offset=0))


def _build_program(host):
    import concourse.bacc as bacc
    import concourse.mybir as mybir
    import concourse.tile as tile

    fp32 = mybir.dt.float32
    bft = mybir.dt.bfloat16
    i16 = mybir.dt.int16
    fp8 = mybir.dt.float8e4
    Alu = mybir.AluOpType
    Act = mybir.ActivationFunctionType

    g_prof = host["g_prof"]
    L_LO, L_HI = host["L_LO"], host["L_HI"]

    nc = bacc.Bacc("TRN2")

    # Pre-place one activation-table load that covers every func we use, so
    # the compile-time fixpoint pass never needs to thrash between sets.
    from concourse.hw_specs import get_activation_tables
    _tabs = list(get_activation_tables(nc.m.arch).items())
    _need = {Act.Exp, Act.Ln, Act.Copy, Act.Relu}
    _set_id = next(i for i, (_n, fns) in enumerate(_tabs) if _need <= fns)

    xT_d = nc.dram_tensor("xT", [128, NP2], bft, kind="ExternalInput")
    xown_d = nc.dram_tensor("x_own", [OWN, D], fp32, kind="ExternalInput")
    Wp_d = nc.dram_tensor("Wp", [128, 144], bft, kind="ExternalInput")
    I128_d = nc.dram_tensor("I128", [128, 128], bft, kind="ExternalInput")
    W1_d = nc.dram_tensor("W1", [128, 256], bft, kind="ExternalInput")
    W2_d = nc.dram_tensor("W2", [256, 128], bft, kind="ExternalInput")
    b1c_d = nc.dram_tensor("b1col", [128, 2], fp32, kind="ExternalInput")
    gl_d = {}
    if not host["triv_bgat"]:
        gl_d["bgat"] = nc.dram_tensor("bgat_r", [128, 128], fp32, kind="ExternalInput")
    if not host["triv_bff2"]:
        gl_d["bff2"] = nc.dram_tensor("bff2_r", [128, 128], fp32, kind="ExternalInput")
    if not host["triv_gb1"]:
        gl_d["g1"] = nc.dram_tensor("g1_r", [128, 128], fp32, kind="ExternalInput")
        gl_d["b1"] = nc.dram_tensor("b1_r", [128, 128], fp32, kind="ExternalInput")
    if not host["triv_gb2"]:
        gl_d["g2"] = nc.dram_tensor("g2_r", [128, 128], fp32, kind="ExternalInput")
        gl_d["b2"] = nc.dram_tensor("b2_r", [128, 128], fp32, kind="ExternalInput")

    st_d = []
    for sname, Ls in (("lo", L_LO), ("hi", L_HI)):
        st_d.append({
            "idx": nc.dram_tensor(f"idx_{sname}", [128, (Ls // CHUNK) * 512], i16,
                                  kind="ExternalInput"),
            "S": nc.dram_tensor(f"S_{sname}", [128, (Ls // CHUNK) * GPC * 64],
                                i16, kind="ExternalInput"),
            "L": Ls,
        })

    h_d = nc.dram_tensor("h_scratch", [NP2, ROW], bft, kind="Internal")
    z_d = nc.dram_tensor("z", [OWN, D], fp32, kind="ExternalOutput")

    NT2 = NP2 // 128                  # 393 node tiles
    PADT = OWN // 128                 # tile 49 == the zero-pad block
    GT = 3                            # node tiles per psum bank
    SW = 12                           # node tiles per stage flush / x DMA

    with tile.TileContext(nc) as tc:
        nc.scalar.add_instruction(mybir.InstLoadActFuncSet(
            name=nc.get_next_instruction_name(), ins=[], outs=[],
            act_func_set_id=_set_id))
        # ================= consts =================
        cpool = tc.alloc_tile_pool(name="consts", bufs=1)
        Wp_s = cpool.tile([128, 144], bft)
        nc.sync.dma_start(out=Wp_s[:], in_=Wp_d[:])
        I128_s = cpool.tile([128, 128], bft)
        nc.sync.dma_start(out=I128_s[:], in_=I128_d[:])
        W1_s = cpool.tile([128, 256], bft)
        nc.sync.dma_start(out=W1_s[:], in_=W1_d[:])
        W2_s = cpool.tile([256 // 2, 2, 128], bft)
        nc.sync.dma_start(out=W2_s[:],
                          in_=W2_d[:].rearrange("(k h) f -> h k f", k=2))
        b1c_s = cpool.tile([128, 2], fp32)
        nc.sync.dma_start(out=b1c_s[:], in_=b1c_d[:])
        gl_s = {}
        for k, dref in gl_d.items():
            gl_s[k] = cpool.tile([128, 128], fp32, tag=f"gl_{k}")
            nc.sync.dma_start(out=gl_s[k][:], in_=dref[:])
        eps_s = cpool.tile([128, 1], fp32)
        nc.vector.memset(eps_s[:], LN_EPS)
        # two fixed stage buffers (junk cols stay zero)
        stgpool = tc.alloc_tile_pool(name="stg", bufs=1)
        stg = []
        for i in range(2):
            s = stgpool.tile([128, SW, ROW], bft, tag=f"stage{i}")
            nc.vector.memset(s[:], 0.0)
            stg.append(s)

        # ================= phase A =================
        with tc.tile_pool(name="pA", bufs=4) as pA, \
             tc.tile_pool(name="psA", bufs=4, space="PSUM") as psA:
            xt = None
            for tg in range((NT2 + GT - 1) // GT):
                t0 = tg * GT
                ntl = min(GT, NT2 - t0)
                if t0 % SW == 0:
                    nxb = min(SW, NT2 - t0)
                    xt = pA.tile([128, SW * 128], bft, tag="xt")
                    nc.sync.dma_start(out=xt[:, :nxb * 128],
                                      in_=xT_d[:, t0 * 128:(t0 + nxb) * 128])
                ps = psA.tile([128, GT, 144], fp32, tag="psA")
                for j in range(ntl):
                    jo = (t0 % SW) + j
                    nc.tensor.matmul(ps[:, j, :],
                                     lhsT=xt[:, jo * 128:(jo + 1) * 128],
                                     rhs=Wp_s[:], start=True, stop=True)
                sb = stg[(t0 // SW) % 2]
                j0 = t0 % SW
                if tg % 2 == 0:
                    nc.scalar.activation(out=sb[:, j0:j0 + ntl, 0:144],
                                         in_=ps[:, :ntl, :], func=Act.Copy)
                else:
                    nc.vector.tensor_copy(out=sb[:, j0:j0 + ntl, 0:144],
                                          in_=ps[:, :ntl, :])
                if j0 + ntl == SW or t0 + ntl == NT2:
                    nf = j0 + ntl
                    r0 = (t0 + ntl - nf) * 128
                    nc.sync.dma_start(
                        out=h_d[r0:r0 + nf * 128, :].rearrange(
                            "(j n) d -> n j d", j=nf),
                        in_=sb[:, :nf, :])
        tc.strict_bb_all_engine_barrier()
        stgpool.release()

        # ================= phases B + C =================
        h_lo = h_d[0:LO_LIM, :]
        h_hi = h_d[LO_LIM:NP2, :]
        ad_tab = h_d[:, C_AD:C_AD + 16]   # [NP2, 16] at 512B pitch
        starts = np.zeros((NBLK, 2), dtype=np.int64)
        for s in range(2):
            starts[1:, s] = np.cumsum(g_prof[:-1, s])

        pB = tc.alloc_tile_pool(name="pB", bufs=5)
        pBs = tc.alloc_tile_pool(name="pBsmall", bufs=6)
        psB = tc.alloc_tile_pool(name="psB", bufs=4, space="PSUM")
        pC = tc.alloc_tile_pool(name="pC", bufs=2)
        psC = tc.alloc_tile_pool(name="psC", bufs=1, space="PSUM")

        chunk_tiles = [{}, {}]

        def emit_chunk(s, k, prefetch=True):
            if k in chunk_tiles[s]:
                return chunk_tiles[s][k]
            sd = st_d[s]
            idx = pBs.tile([128, 512], i16, tag="idx")
            nc.sync.dma_start(out=idx[:],
                              in_=sd["idx"][:, k * 512:(k + 1) * 512])
            Sg = pB.tile([128, GPC, 64], i16, tag="Sg")
            nc.sync.dma_start(out=Sg[:],
                              in_=sd["S"][:, k * GPC * 64:(k + 1) * GPC * 64]
                              .rearrange("p (g c) -> p g c", c=64))
            hrow = pB.tile([128, GPC, ROW], bft, tag="h")
            nc.gpsimd.dma_gather(hrow[:], h_lo if s == 0 else h_hi,
                                 idx[:, 0:256], CHUNK, CHUNK, ROW,
                                 single_packet=False)
            meta = pB.tile([128, GPC, 16], bft, tag="m")
            _dma_gather_raw(nc.gpsimd, meta[:], ad_tab, idx[:, 256:512],
                            CHUNK, 16, ROW)
            # eL = a_src[src] + a_dst[dst]
            eL = pBs.tile([128, GPC, 8], bft, tag="eL")
            nc.vector.tensor_tensor(out=eL[:], in0=hrow[:, :, C_AS:C_AS + 8],
                                    in1=meta[:, :, 0:8], op=Alu.add)
            eL2 = pBs.tile([128, GPC, 8], bft, tag="eL2")
            nc.vector.scalar_tensor_tensor(out=eL2[:], in0=eL[:], scalar=0.2,
                                           in1=eL[:], op0=Alu.mult, op1=Alu.max)
            # p over the spent a_src cols; msg in place over h
            nc.scalar.activation(out=hrow[:, :, 128:136], in_=eL2[:],
                                 func=Act.Exp)
            pexp = pB.tile([128, GPC, 128], bft, tag="px")
            nc.scalar.activation(
                out=pexp[:].rearrange("p g (h f) -> p g h f", f=F),
                in_=eL2[:].to_broadcast([128, GPC, 8, F]), func=Act.Exp)
            nc.vector.tensor_tensor(out=hrow[:, :, 0:128],
                                    in0=hrow[:, :, 0:128], in1=pexp[:],
                                    op=Alu.mult)
            res = {"S": Sg, "msgp": hrow}
            chunk_tiles[s][k] = res
            # prefetch one chunk ahead (non-recursive) so its gathers are in
            # the instruction stream before this chunk's consumers
            if prefetch and (k + 1) * CHUNK < sd["L"]:
                emit_chunk(s, k + 1, prefetch=False)
            return res

        for b in range(NBLK):
            ps_blk = psB.tile([128, 136], fp32, tag="blk")
            tot = int(g_prof[b, 0] + g_prof[b, 1])
            done = 0
            for s in range(2):
                for gi in range(int(g_prof[b, s])):
                    gg = int(starts[b, s]) + gi
                    ct = emit_chunk(s, gg // GPC)
                    gl = gg % GPC
                    nc.tensor.matmul(
                        ps_blk[:],
                        lhsT=ct["S"][:, gl, :].bitcast(fp8),
                        rhs=ct["msgp"][:, gl, 0:136],
                        start=(done == 0), stop=(done == tot - 1))
                    done += 1
            rec = pBs.tile([128, 8], fp32, tag="rec")
            nc.vector.reciprocal(out=rec[:], in_=ps_blk[:, 128:136])
            gt = pC.tile([128, 128], fp32, tag="gt")
            nc.vector.tensor_tensor(
                out=gt[:].rearrange("p (h f) -> p h f", f=F),
                in0=ps_blk[:, 0:128].rearrange("p (h f) -> p h f", f=F),
                in1=rec[:].to_broadcast([128, 8, F]), op=Alu.mult)
            if not host["triv_bgat"]:
                nc.vector.tensor_tensor(out=gt[:], in0=gt[:], in1=gl_s["bgat"][:],
                                        op=Alu.add)
            # ---- phase C for tile b ----
            xo = pC.tile([128, 128], fp32, tag="xo")
            nc.sync.dma_start(out=xo[:], in_=xown_d[b * 128:(b + 1) * 128, :])
            t1 = pC.tile([128, 128], fp32, tag="t1")
            nc.vector.tensor_tensor(out=t1[:], in0=xo[:], in1=gt[:], op=Alu.add)

            def layer_norm(tin, g_key, b_key, triv, tagp):
                bst = pBs.tile([128, 6], fp32, tag=f"bst{tagp}")
                nc.vector.bn_stats(out=bst[:], in_=tin[:])
                mv = pBs.tile([128, 2], fp32, tag=f"mv{tagp}")
                nc.vector.bn_aggr(out=mv[:], in_=bst[:])
                # rstd = exp(-0.5 * ln(var + eps)); stays in the exp/ln set
                nc.scalar.activation(out=mv[:, 1:2], in_=mv[:, 1:2],
                                     func=Act.Ln, bias=eps_s[:])
                nc.scalar.activation(out=mv[:, 1:2], in_=mv[:, 1:2],
                                     func=Act.Exp, scale=-0.5)
                o = pC.tile([128, 128], fp32, tag=f"ln{tagp}")
                nc.vector.tensor_scalar(out=o[:], in0=tin[:],
                                        scalar1=mv[:, 0:1], op0=Alu.subtract,
                                        scalar2=mv[:, 1:2], op1=Alu.mult)
                if not triv:
                    nc.vector.tensor_tensor(out=o[:], in0=o[:], in1=gl_s[g_key][:],
                                            op=Alu.mult)
                    nc.vector.tensor_tensor(out=o[:], in0=o[:], in1=gl_s[b_key][:],
                                            op=Alu.add)
                return o

            u = layer_norm(t1, "g1", "b1", host["triv_gb1"], "1")
            u_bf = pC.tile([128, 128], bft, tag="ubf")
            nc.scalar.activation(out=u_bf[:], in_=u[:], func=Act.Copy)
            uT_ps = psC.tile([128, 128], bft, tag="uT")
            nc.tensor.transpose(uT_ps[:], in_=u_bf[:], identity=I128_s[:])
            uT = pC.tile([128, 128], bft, tag="uTs")
            nc.scalar.activation(out=uT[:], in_=uT_ps[:], func=Act.Copy)
            f1ps = psC.tile([128, 2, 128], fp32, tag="f1")
            for j in range(2):
                nc.tensor.matmul(f1ps[:, j, :], lhsT=W1_s[:, j * 128:(j + 1) * 128],
                                 rhs=uT[:], start=True, stop=True)
            r1 = pC.tile([128, 2, 128], bft, tag="r1")
            for j in range(2):
                nc.scalar.activation(out=r1[:, j, :], in_=f1ps[:, j, :],
                                     func=Act.Relu, bias=b1c_s[:, j:j + 1])
            zps = psC.tile([128, 128], fp32, tag="zp")
            for j in range(2):
                nc.tensor.matmul(zps[:], lhsT=r1[:, j, :], rhs=W2_s[:, j, :],
                                 start=(j == 0), stop=(j == 1))
            t2 = pC.tile([128, 128], fp32, tag="t2")
            nc.vector.tensor_tensor(out=t2[:], in0=u[:], in1=zps[:], op=Alu.add)
            if not host["triv_bff2"]:
                nc.vector.tensor_tensor(out=t2[:], in0=t2[:], in1=gl_s["bff2"][:],
                                        op=Alu.add)
            zt = layer_norm(t2, "g2", "b2", host["triv_gb2"], "2")
            nc.sync.dma_start(out=z_d[b * 128:(b + 1) * 128, :], in_=zt[:])

        for p in (psC, pC, psB, pBs, pB):
            p.release()
        cpool.release()

    nc.compile()
    return nc


def kernel(**inputs):
    from concourse.bass_utils import run_bass_kernel_spmd
    import os

    host = _build_host_data(inputs)
    nc = _build_program(host)

    in_maps = []
    for c in range(NCORES):
        m = {
            "xT": host["xT"][c],
            "x_own": host["x_own"][c],
            "Wp": host["Wp"], "I128": host["I128"],
            "W1": host["W1"], "W2": host["W2"], "b1col": host["b1col"],
        }
        if not host["triv_bgat"]:
            m["bgat_r"] = np.tile(host["bias_gat"].reshape(1, -1), (128, 1))
        if not host["triv_bff2"]:
            m["bff2_r"] = np.tile(host["b_ff2"].reshape(1, -1), (128, 1))
        if not host["triv_gb1"]:
            m["g1_r"] = np.tile(host["gamma1"].reshape(1, -1), (128, 1))
            m["b1_r"] = np.tile(host["beta1"].reshape(1, -1), (128, 1))
        if not host["triv_gb2"]:
            m["g2_r"] = np.tile(host["gamma2"].reshape(1, -1), (128, 1))
            m["b2_r"] = np.tile(host["beta2"].reshape(1, -1), (128, 1))
        for s, sname in ((0, "lo"), (1, "hi")):
            m[f"idx_{sname}"] = host["per_core"][c][s]["idx"]
            m[f"S_{sname}"] = host["per_core"][c][s]["S"]
        in_maps.append(m)

    trace = bool(int(os.environ.get("GAT_TRACE", "0")))
    res = run_bass_kernel_spmd(nc, in_maps, core_ids=list(range(NCORES)),
                               trace=trace)
    if trace and res.exec_time_ns:
        print(f"HW exec time: {res.exec_time_ns} ns")
    if bool(int(os.environ.get("GAT_TIME", "0"))):
        try:
            from concourse.timeline_sim import TimelineSim
            ts = TimelineSim(nc)
            dur = ts.simulate()
            print(f"HW exec time: {dur:.0f} ns (cost-model timeline estimate)")
        except Exception as e:
            print("timeline sim failed:", e)

    out = np.zeros((N, D), np.float32)
    for c in range(NCORES):
        lo_n = OWN * c
        hi_n = min(OWN * (c + 1), N)
        out[lo_n:hi_n] = res.results[c]["z"][: hi_n - lo_n]
    return out


# revision 31
# speedup vs baseline: 1.6670x; 1.0437x over previous
"""Trainium2 Bass kernel for a GAT block (GATConv + LN + FFN + LN).

Self-contained: builds per-core shards on the host, compiles one SPMD Bass
program, runs it on 8 NeuronCores via run_bass_kernel_spmd, and reassembles
the full [50000, 128] output.

Per-core scheme (core c of 8, nodes permuted own|zeropad|rest):
  Phase A: for all 50304 (padded) rows compute [h | a_src | a_dst] =
           x @ [W | W@Asrc | W@Adst] on PE; store 512-byte node rows
           [h(128) | a_src(8) | a_dst(8) | onehot_fp8(64 bf16 slots) | pad]
           to core-local DRAM. The onehot block is a constant identity
           pattern (row r holds onehot(r % 128) in fp8) that later serves
           as ready-made scatter-matmul weights.
  Phase B: edges with dst owned by the core (incl self-loops), grouped by
           128-node dst block, split lo/hi on the 32K int16 gather-index
           limit, padded per (block, stream) to 128-edge granules with a
           shared max-over-cores profile so all cores run one program.
           Per 4096-edge chunk: gather#1 512B src rows (h + a_src);
           gather#2 256B dst meta half-rows (a_dst + fp8 onehot = S);
           eL = a_src + a_dst; p = exp(leaky_relu(eL)); pexp = head-expand
           of p on the Act engine; msg = h * pexp (DVE 2x); per granule:
           psum[block] += S_fp8^T-matmul [msg | p].
  Phase C: g = agg/denom; u = LN(x + g); ff = relu(u@W1 + b1)@W2 + b2;
           z = LN(u + ff). rstd via exp(-0.5*ln(var+eps)) keeps every
           activation in one table set (no LoadActFuncSet churn).
"""
import numpy as np
import ml_dtypes

N = 50000
NCORES = 8
OWN = 6272             # nodes per core (49 tiles of 128)
ZPAD = 128             # zero rows after own block (pad-edge target)
NPG = OWN * NCORES     # globally padded node count (50176)
NP2 = NPG + ZPAD       # per-core row count (50304)
BLK = 128
NBLK = OWN // BLK      # 49
GR = 128               # edges per granule
CHUNK = 4096
GPC = CHUNK // GR      # 32
LO_LIM = 1 << 15
H, F, D = 8, 16, 128
ROW = 256              # bf16 cols per node row (512 bytes)
C_AS = 128             # a_src col
C_AD = 136             # a_dst col
C_OH = 144             # onehot (fp8-as-bf16) col, 64 cols
LN_EPS = 1e-5

bf16 = ml_dtypes.bfloat16
f8 = ml_dtypes.float8_e4m3fn


def _wrapc(idx):
    """Per-chunk 16-wrap: [CHUNK] int -> [128, CHUNK//16] int16."""
    w = idx.reshape(CHUNK // 16, 16).T.astype(np.int16)
    return np.tile(w, (8, 1))


def _bfr(x):
    return np.ascontiguousarray(x, dtype=np.float32).astype(bf16)


def _build_host_data(inputs):
    x = np.asarray(inputs["x"], np.float32)
    W = np.asarray(inputs["W_gat"], np.float32)
    att_src = np.asarray(inputs["att_src"], np.float32)
    att_dst = np.asarray(inputs["att_dst"], np.float32)
    ei = np.asarray(inputs["edge_index"])

    src = ei[0].astype(np.int64)
    dst = ei[1].astype(np.int64)
    loops = np.arange(N, dtype=np.int64)
    src = np.concatenate([src, loops])
    dst = np.concatenate([dst, loops])

    # per-core row index of global padded node g:
    #   own -> [0, OWN); zeros -> [OWN, OWN+ZPAD); rest keeps order after
    def inv_row(c, g):
        own = (g >= OWN * c) & (g < OWN * (c + 1))
        r = np.where(own, g - OWN * c,
                     OWN + ZPAD + np.where(g < OWN * c, g, g - OWN))
        return r

    # per (core, block, stream) counts on row-mapped gather indices
    counts = np.zeros((NCORES, NBLK, 2), dtype=np.int64)
    core_edges = []
    for c in range(NCORES):
        m = (dst >= OWN * c) & (dst < min(OWN * (c + 1), N))
        s_g = inv_row(c, src[m])
        d_l = dst[m] - OWN * c
        blk = d_l // BLK
        lo = s_g < LO_LIM
        core_edges.append((s_g, d_l, blk, lo))
        for b in range(NBLK):
            mb = blk == b
            counts[c, b, 0] = np.sum(mb & lo)
            counts[c, b, 1] = np.sum(mb & ~lo)

    g_prof = np.ceil(counts.max(axis=0) / GR).astype(np.int64)   # [NBLK, 2]
    L = [int(g_prof[:, s].sum()) * GR for s in range(2)]
    for s in range(2):
        pad = (-L[s]) % CHUNK
        g_prof[NBLK - 1, s] += pad // GR
        L[s] += pad
    L_LO, L_HI = L

    per_core = []
    for c in range(NCORES):
        s_g, d_l, blk, lo = core_edges[c]
        streams = []
        for sidx in range(2):
            mm = lo if sidx == 0 else ~lo
            Ls = L[sidx]
            gidx = np.zeros(Ls, dtype=np.int64)
            aidx = np.full(Ls, OWN, dtype=np.int64)   # pads -> zero row
            dl = np.full(Ls, -1, dtype=np.int64)      # pads -> no S entry
            pos = 0
            for b in range(NBLK):
                mb = (blk == b) & mm
                k = int(np.sum(mb))
                cap = int(g_prof[b, sidx]) * GR
                gidx[pos:pos + k] = s_g[mb] - (0 if sidx == 0 else LO_LIM)
                aidx[pos:pos + k] = d_l[mb]
                dl[pos:pos + k] = d_l[mb] % BLK
                pos += cap
            # pack [gidx | aidx] wrapped per chunk: [128, nch*512] i16
            nch = Ls // CHUNK
            pk = np.zeros((128, nch * 512), dtype=np.int16)
            for k in range(nch):
                pk[:, k * 512:k * 512 + 256] = _wrapc(gidx[k * CHUNK:(k + 1) * CHUNK])
                pk[:, k * 512 + 256:(k + 1) * 512] = _wrapc(aidx[k * CHUNK:(k + 1) * CHUNK])
            # S stream: per slot a 128-byte fp8 onehot(dl) row, laid out
            # [128 partitions(edge%128), nch*GPC*128 bytes] -> i16 view
            SB = np.zeros((Ls, 128), dtype=np.uint8)
            real = dl >= 0
            SB[np.nonzero(real)[0], dl[real]] = 0x38          # fp8 e4m3 1.0
            SB = SB.reshape(nch, GPC, 128, 128).transpose(2, 0, 1, 3)
            SB = np.ascontiguousarray(SB.reshape(128, nch * GPC * 128))
            streams.append({"idx": np.ascontiguousarray(pk),
                            "S": SB.view(np.int16)})
        per_core.append(streams)

    # weights: Wp = [W | W@Asrc | W@Adst]  -> [128, 144]
    Asrc = np.zeros((D, H), np.float32)
    Adst = np.zeros((D, H), np.float32)
    for h in range(H):
        Asrc[h * F:(h + 1) * F, h] = att_src[h]
        Adst[h * F:(h + 1) * F, h] = att_dst[h]
    Wp = _bfr(np.concatenate([W, W @ Asrc, W @ Adst], axis=1))   # [128, 144]
    I128 = _bfr(np.eye(128, dtype=np.float32))

    xp = np.zeros((NP2, D), np.float32)
    xp[:N] = x                         # global padded layout first
    xT_per_core = []
    x_own_per_core = []
    for c in range(NCORES):
        rows = np.zeros((NP2, D), np.float32)
        rows[0:OWN] = xp[OWN * c: OWN * (c + 1)]
        rest = np.concatenate([xp[: OWN * c], xp[OWN * (c + 1): NPG]])
        rows[OWN + ZPAD:] = rest
        xT_per_core.append(np.ascontiguousarray(rows.T.astype(f8)))
        x_own_per_core.append(np.ascontiguousarray(xp[OWN * c: OWN * (c + 1)]))

    host = {
        "g_prof": g_prof, "L_LO": L_LO, "L_HI": L_HI,
        "per_core": per_core, "xT": xT_per_core, "x_own": x_own_per_core,
        "Wp": Wp, "I128": I128,
        "W1": _bfr(np.asarray(inputs["w_ff1"], np.float32)),
        "W2": _bfr(np.asarray(inputs["w_ff2"], np.float32)),
        "b1col": np.ascontiguousarray(
            np.asarray(inputs["b_ff1"], np.float32).reshape(2, 128).T),
    }
    host["bias_gat"] = np.asarray(inputs["bias_gat"], np.float32)
    host["b_ff2"] = np.asarray(inputs["b_ff2"], np.float32)
    for nm in ("gamma1", "beta1", "gamma2", "beta2"):
        host[nm] = np.asarray(inputs[nm], np.float32)
    host["triv_gb1"] = bool(np.all(host["gamma1"] == 1) and np.all(host["beta1"] == 0))
    host["triv_gb2"] = bool(np.all(host["gamma2"] == 1) and np.all(host["beta2"] == 0))
    host["triv_bgat"] = bool(np.all(host["bias_gat"] == 0))
    host["triv_bff2"] = bool(np.all(host["b_ff2"] == 0))
    return host


def _dma_gather_raw(eng, out_ap, in_ap, idxs_ap, num_idxs, elem_size,
                    elem_step):
    """dma_gather with elem_size below the 256-byte API floor (the floor is a
    transpose-mode restriction; plain row gathers take any size whose row
    pitch is a 256B multiple)."""
    import concourse.mybir as mybir
    from concourse import ap_utils

    assert idxs_ap.dtype == mybir.dt.int16
    assert in_ap.dtype == out_ap.dtype
    assert ap_utils.ap_is_contiguous(out_ap.ap[1:])
    assert ap_utils.ap_is_contiguous(idxs_ap.ap[1:])
    assert in_ap.ap[-1][1] == out_ap.ap[-1][1] == elem_size
    assert in_ap.ap[0][0] == elem_step
    stride_bytes = elem_step * mybir.dt.size(in_ap.dtype)
    assert stride_bytes % 256 == 0
    _in_ap = eng.lower_ap_dma(in_ap, for_custom_bir_dma=True)
    _idxs_ap = eng.lower_ap(idxs_ap)
    _out_ap = eng.lower_ap(out_ap)
    return eng.add_instruction(mybir.InstDMAGatherAnt(
        name=eng.bass.get_next_instruction_name(),
        ins=[*_in_ap, _idxs_ap, eng.lower_val_access(eng.to_reg(num_idxs))],
        outs=[_out_ap],
        transpose=False, num_idxs=num_idxs, elem_size=elem_size,
        stride_bytes_256=stride_bytes // 256, gen_mode=0,
        single_packet=False, queue_num=0,
        sbuf_tokens_per_rank=0, sbuf_free_dim_per_rank=0,
        sbuf_free_dim_pad_per_rank=0, sbuf_byte_offset=0))


def _build_program(host):
    import concourse.bacc as bacc
    import concourse.mybir as mybir
    import concourse.tile as tile

    fp32 = mybir.dt.float32
    bft = mybir.dt.bfloat16
    i16 = mybir.dt.int16
    fp8 = mybir.dt.float8e4
    Alu = mybir.AluOpType
    Act = mybir.ActivationFunctionType

    g_prof = host["g_prof"]
    L_LO, L_HI = host["L_LO"], host["L_HI"]

    nc = bacc.Bacc("TRN2")

    # Pre-place one activation-table load that covers every func we use, so
    # the compile-time fixpoint pass never needs to thrash between sets.
    from concourse.hw_specs import get_activation_tables
    _tabs = list(get_activation_tables(nc.m.arch).items())
    _need = {Act.Exp, Act.Ln, Act.Copy, Act.Relu}
    _set_id = next(i for i, (_n, fns) in enumerate(_tabs) if _need <= fns)

    xT_d = nc.dram_tensor("xT", [128, NP2], fp8, kind="ExternalInput")
    xown_d = nc.dram_tensor("x_own", [OWN, D], fp32, kind="ExternalInput")
    Wp_d = nc.dram_tensor("Wp", [128, 144], bft, kind="ExternalInput")
    I128_d = nc.dram_tensor("I128", [128, 128], bft, kind="ExternalInput")
    W1_d = nc.dram_tensor("W1", [128, 256], bft, kind="ExternalInput")
    W2_d = nc.dram_tensor("W2", [256, 128], bft, kind="ExternalInput")
    b1c_d = nc.dram_tensor("b1col", [128, 2], fp32, kind="ExternalInput")
    gl_d = {}
    if not host["triv_bgat"]:
        gl_d["bgat"] = nc.dram_tensor("bgat_r", [128, 128], fp32, kind="ExternalInput")
    if not host["triv_bff2"]:
        gl_d["bff2"] = nc.dram_tensor("bff2_r", [128, 128], fp32, kind="ExternalInput")
    if not host["triv_gb1"]:
        gl_d["g1"] = nc.dram_tensor("g1_r", [128, 128], fp32, kind="ExternalInput")
        gl_d["b1"] = nc.dram_tensor("b1_r", [128, 128], fp32, kind="ExternalInput")
    if not host["triv_gb2"]:
        gl_d["g2"] = nc.dram_tensor("g2_r", [128, 128], fp32, kind="ExternalInput")
        gl_d["b2"] = nc.dram_tensor("b2_r", [128, 128], fp32, kind="ExternalInput")

    st_d = []
    for sname, Ls in (("lo", L_LO), ("hi", L_HI)):
        st_d.append({
            "idx": nc.dram_tensor(f"idx_{sname}", [128, (Ls // CHUNK) * 512], i16,
                                  kind="ExternalInput"),
            "S": nc.dram_tensor(f"S_{sname}", [128, (Ls // CHUNK) * GPC * 64],
                                i16, kind="ExternalInput"),
            "L": Ls,
        })

    h_d = nc.dram_tensor("h_scratch", [NP2, ROW], bft, kind="Internal")
    z_d = nc.dram_tensor("z", [OWN, D], fp32, kind="ExternalOutput")

    NT2 = NP2 // 128                  # 393 node tiles
    PADT = OWN // 128                 # tile 49 == the zero-pad block
    GT = 3                            # node tiles per psum bank
    SW = 12                           # node tiles per stage flush / x DMA

    with tile.TileContext(nc) as tc:
        nc.scalar.add_instruction(mybir.InstLoadActFuncSet(
            name=nc.get_next_instruction_name(), ins=[], outs=[],
            act_func_set_id=_set_id))
        # ================= consts =================
        cpool = tc.alloc_tile_pool(name="consts", bufs=1)
        Wp_s = cpool.tile([128, 144], bft)
        nc.sync.dma_start(out=Wp_s[:], in_=Wp_d[:])
        I128_s = cpool.tile([128, 128], bft)
        nc.sync.dma_start(out=I128_s[:], in_=I128_d[:])
        W1_s = cpool.tile([128, 256], bft)
        nc.sync.dma_start(out=W1_s[:], in_=W1_d[:])
        W2_s = cpool.tile([256 // 2, 2, 128], bft)
        nc.sync.dma_start(out=W2_s[:],
                          in_=W2_d[:].rearrange("(k h) f -> h k f", k=2))
        b1c_s = cpool.tile([128, 2], fp32)
        nc.sync.dma_start(out=b1c_s[:], in_=b1c_d[:])
        gl_s = {}
        for k, dref in gl_d.items():
            gl_s[k] = cpool.tile([128, 128], fp32, tag=f"gl_{k}")
            nc.sync.dma_start(out=gl_s[k][:], in_=dref[:])
        eps_s = cpool.tile([128, 1], fp32)
        nc.vector.memset(eps_s[:], LN_EPS)
        # two fixed stage buffers (junk cols stay zero)
        stgpool = tc.alloc_tile_pool(name="stg", bufs=1)
        stg = []
        for i in range(2):
            s = stgpool.tile([128, SW, ROW], bft, tag=f"stage{i}")
            nc.vector.memset(s[:], 0.0)
            stg.append(s)

        # ================= phase A =================
        with tc.tile_pool(name="pA", bufs=4) as pA, \
             tc.tile_pool(name="psA", bufs=4, space="PSUM") as psA:
            xt = None
            for tg in range((NT2 + GT - 1) // GT):
                t0 = tg * GT
                ntl = min(GT, NT2 - t0)
                if t0 % SW == 0:
                    nxb = min(SW, NT2 - t0)
                    xt = pA.tile([128, SW * 128], fp8, tag="xt")
                    nc.sync.dma_start(out=xt[:, :nxb * 128],
                                      in_=xT_d[:, t0 * 128:(t0 + nxb) * 128])
                ps = psA.tile([128, GT, 144], fp32, tag="psA")
                for j in range(ntl):
                    jo = (t0 % SW) + j
                    nc.tensor.matmul(ps[:, j, :],
                                     lhsT=xt[:, jo * 128:(jo + 1) * 128],
                                     rhs=Wp_s[:], start=True, stop=True)
                sb = stg[(t0 // SW) % 2]
                j0 = t0 % SW
                if tg % 2 == 0:
                    nc.scalar.activation(out=sb[:, j0:j0 + ntl, 0:144],
                                         in_=ps[:, :ntl, :], func=Act.Copy)
                else:
                    nc.vector.tensor_copy(out=sb[:, j0:j0 + ntl, 0:144],
                                          in_=ps[:, :ntl, :])
                if j0 + ntl == SW or t0 + ntl == NT2:
                    nf = j0 + ntl
                    r0 = (t0 + ntl - nf) * 128
                    nc.sync.dma_start(
                        out=h_d[r0:r0 + nf * 128, :].rearrange(
                            "(j n) d -> n j d", j=nf),
                        in_=sb[:, :nf, :])
        tc.strict_bb_all_engine_barrier()
        stgpool.release()

        # ================= phases B + C =================
        h_lo = h_d[0:LO_LIM, :]
        h_hi = h_d[LO_LIM:NP2, :]
        ad_tab = h_d[:, C_AD:C_AD + 16]   # [NP2, 16] at 512B pitch
        starts = np.zeros((NBLK, 2), dtype=np.int64)
        for s in range(2):
            starts[1:, s] = np.cumsum(g_prof[:-1, s])

        pB = tc.alloc_tile_pool(name="pB", bufs=5)
        pBs = tc.alloc_tile_pool(name="pBsmall", bufs=6)
        psB = tc.alloc_tile_pool(name="psB", bufs=4, space="PSUM")
        pC = tc.alloc_tile_pool(name="pC", bufs=2)
        psC = tc.alloc_tile_pool(name="psC", bufs=1, space="PSUM")

        chunk_tiles = [{}, {}]

        def emit_chunk(s, k, prefetch=True):
            if k in chunk_tiles[s]:
                return chunk_tiles[s][k]
            sd = st_d[s]
            idx = pBs.tile([128, 512], i16, tag="idx")
            nc.sync.dma_start(out=idx[:],
                              in_=sd["idx"][:, k * 512:(k + 1) * 512])
            Sg = pB.tile([128, GPC, 64], i16, tag="Sg")
            nc.sync.dma_start(out=Sg[:],
                              in_=sd["S"][:, k * GPC * 64:(k + 1) * GPC * 64]
                              .rearrange("p (g c) -> p g c", c=64))
            hrow = pB.tile([128, GPC, ROW], bft, tag="h")
            nc.gpsimd.dma_gather(hrow[:], h_lo if s == 0 else h_hi,
                                 idx[:, 0:256], CHUNK, CHUNK, ROW,
                                 single_packet=False)
            meta = pB.tile([128, GPC, 16], bft, tag="m")
            _dma_gather_raw(nc.gpsimd, meta[:], ad_tab, idx[:, 256:512],
                            CHUNK, 16, ROW)
            # eL = a_src[src] + a_dst[dst]
            eL = pBs.tile([128, GPC, 8], bft, tag="eL")
            nc.vector.tensor_tensor(out=eL[:], in0=hrow[:, :, C_AS:C_AS + 8],
                                    in1=meta[:, :, 0:8], op=Alu.add)
            eL2 = pBs.tile([128, GPC, 8], bft, tag="eL2")
            nc.vector.scalar_tensor_tensor(out=eL2[:], in0=eL[:], scalar=0.2,
                                           in1=eL[:], op0=Alu.mult, op1=Alu.max)
            # p over the spent a_src cols; msg in place over h
            nc.scalar.activation(out=hrow[:, :, 128:136], in_=eL2[:],
                                 func=Act.Exp)
            pexp = pB.tile([128, GPC, 128], bft, tag="px")
            nc.scalar.activation(
                out=pexp[:].rearrange("p g (h f) -> p g h f", f=F),
                in_=eL2[:].to_broadcast([128, GPC, 8, F]), func=Act.Exp)
            nc.vector.tensor_tensor(out=hrow[:, :, 0:128],
                                    in0=hrow[:, :, 0:128], in1=pexp[:],
                                    op=Alu.mult)
            res = {"S": Sg, "msgp": hrow}
            chunk_tiles[s][k] = res
            # prefetch one chunk ahead (non-recursive) so its gathers are in
            # the instruction stream before this chunk's consumers
            if prefetch and (k + 1) * CHUNK < sd["L"]:
                emit_chunk(s, k + 1, prefetch=False)
            return res

        for b in range(NBLK):
            ps_blk = psB.tile([128, 136], fp32, tag="blk")
            tot = int(g_prof[b, 0] + g_prof[b, 1])
            done = 0
            for s in range(2):
                for gi in range(int(g_prof[b, s])):
                    gg = int(starts[b, s]) + gi
                    ct = emit_chunk(s, gg // GPC)
                    gl = gg % GPC
                    nc.tensor.matmul(
                        ps_blk[:],
                        lhsT=ct["S"][:, gl, :].bitcast(fp8),
                        rhs=ct["msgp"][:, gl, 0:136],
                        start=(done == 0), stop=(done == tot - 1))
                    done += 1
            rec = pBs.tile([128, 8], fp32, tag="rec")
            nc.vector.reciprocal(out=rec[:], in_=ps_blk[:, 128:136])
            gt = pC.tile([128, 128], fp32, tag="gt")
            nc.vector.tensor_tensor(
                out=gt[:].rearrange("p (h f) -> p h f", f=F),
                in0=ps_blk[:, 0:128].rearrange("p (h f) -> p h f", f=F),
                in1=rec[:].to_broadcast([128, 8, F]), op=Alu.mult)
            if not host["triv_bgat"]:
                nc.vector.tensor_tensor(out=gt[:], in0=gt[:], in1=gl_s["bgat"][:],
                                        op=Alu.add)
            # ---- phase C for tile b ----
            xo = pC.tile([128, 128], fp32, tag="xo")
            nc.sync.dma_start(out=xo[:], in_=xown_d[b * 128:(b + 1) * 128, :])
            t1 = pC.tile([128, 128], fp32, tag="t1")
            nc.vector.tensor_tensor(out=t1[:], in0=xo[:], in1=gt[:], op=Alu.add)

            def layer_norm(tin, g_key, b_key, triv, tagp):
                bst = pBs.tile([128, 6], fp32, tag=f"bst{tagp}")
                nc.vector.bn_stats(out=bst[:], in_=tin[:])
                mv = pBs.tile([128, 2], fp32, tag=f"mv{tagp}")
                nc.vector.bn_aggr(out=mv[:], in_=bst[:])
                # rstd = exp(-0.5 * ln(var + eps)); stays in the exp/ln set
                nc.scalar.activation(out=mv[:, 1:2], in_=mv[:, 1:2],
                                     func=Act.Ln, bias=eps_s[:])
                nc.scalar.activation(out=mv[:, 1:2], in_=mv[:, 1:2],
                                     func=Act.Exp, scale=-0.5)
                o = pC.tile([128, 128], fp32, tag=f"ln{tagp}")
                nc.vector.tensor_scalar(out=o[:], in0=tin[:],
                                        scalar1=mv[:, 0:1], op0=Alu.subtract,
                                        scalar2=mv[:, 1:2], op1=Alu.mult)
                if not triv:
                    nc.vector.tensor_tensor(out=o[:], in0=o[:], in1=gl_s[g_key][:],
                                            op=Alu.mult)
                    nc.vector.tensor_tensor(out=o[:], in0=o[:], in1=gl_s[b_key][:],
                                            op=Alu.add)
                return o

            u = layer_norm(t1, "g1", "b1", host["triv_gb1"], "1")
            u_bf = pC.tile([128, 128], bft, tag="ubf")
            nc.scalar.activation(out=u_bf[:], in_=u[:], func=Act.Copy)
            uT_ps = psC.tile([128, 128], bft, tag="uT")
            nc.tensor.transpose(uT_ps[:], in_=u_bf[:], identity=I128_s[:])
            uT = pC.tile([128, 128], bft, tag="uTs")
            nc.scalar.activation(out=uT[:], in_=uT_ps[:], func=Act.Copy)
            f1ps = psC.tile([128, 2, 128], fp32, tag="f1")
            for j in range(2):
                nc.tensor.matmul(f1ps[:, j, :], lhsT=W1_s[:, j * 128:(j + 1) * 128],
                                 rhs=uT[:], start=True, stop=True)
            r1 = pC.tile([128, 2, 128], bft, tag="r1")
            for j in range(2):
                nc.scalar.activation(out=r1[:, j, :], in_=f1ps[:, j, :],
                                     func=Act.Relu, bias=b1c_s[:, j:j + 1])
            zps = psC.tile([128, 128], fp32, tag="zp")
            for j in range(2):
                nc.tensor.matmul(zps[:], lhsT=r1[:, j, :], rhs=W2_s[:, j, :],
                                 start=(j == 0), stop=(j == 1))
            t2 = pC.tile([128, 128], fp32, tag="t2")
            nc.vector.tensor_tensor(out=t2[:], in0=u[:], in1=zps[:], op=Alu.add)
            if not host["triv_bff2"]:
                nc.vector.tensor_tensor(out=t2[:], in0=t2[:], in1=gl_s["bff2"][:],
                                        op=Alu.add)
            zt = layer_norm(t2, "g2", "b2", host["triv_gb2"], "2")
            nc.sync.dma_start(out=z_d[b * 128:(b + 1) * 128, :], in_=zt[:])

        for p in (psC, pC, psB, pBs, pB):
            p.release()
        cpool.release()

    nc.compile()
    return nc


def kernel(**inputs):
    from concourse.bass_utils import run_bass_kernel_spmd
    import os

    host = _build_host_data(inputs)
    nc = _build_program(host)

    in_maps = []
    for c in range(NCORES):
        m = {
            "xT": host["xT"][c],
            "x_own": host["x_own"][c],
            "Wp": host["Wp"], "I128": host["I128"],
            "W1": host["W1"], "W2": host["W2"], "b1col": host["b1col"],
        }
        if not host["triv_bgat"]:
            m["bgat_r"] = np.tile(host["bias_gat"].reshape(1, -1), (128, 1))
        if not host["triv_bff2"]:
            m["bff2_r"] = np.tile(host["b_ff2"].reshape(1, -1), (128, 1))
        if not host["triv_gb1"]:
            m["g1_r"] = np.tile(host["gamma1"].reshape(1, -1), (128, 1))
            m["b1_r"] = np.tile(host["beta1"].reshape(1, -1), (128, 1))
        if not host["triv_gb2"]:
            m["g2_r"] = np.tile(host["gamma2"].reshape(1, -1), (128, 1))
            m["b2_r"] = np.tile(host["beta2"].reshape(1, -1), (128, 1))
        for s, sname in ((0, "lo"), (1, "hi")):
            m[f"idx_{sname}"] = host["per_core"][c][s]["idx"]
            m[f"S_{sname}"] = host["per_core"][c][s]["S"]
        in_maps.append(m)

    trace = bool(int(os.environ.get("GAT_TRACE", "0")))
    res = run_bass_kernel_spmd(nc, in_maps, core_ids=list(range(NCORES)),
                               trace=trace)
    if trace and res.exec_time_ns:
        print(f"HW exec time: {res.exec_time_ns} ns")
    if bool(int(os.environ.get("GAT_TIME", "0"))):
        try:
            from concourse.timeline_sim import TimelineSim
            ts = TimelineSim(nc)
            dur = ts.simulate()
            print(f"HW exec time: {dur:.0f} ns (cost-model timeline estimate)")
        except Exception as e:
            print("timeline sim failed:", e)

    out = np.zeros((N, D), np.float32)
    for c in range(NCORES):
        lo_n = OWN * c
        hi_n = min(OWN * (c + 1), N)
        out[lo_n:hi_n] = res.results[c]["z"][: hi_n - lo_n]
    return out
